# revision 1
# baseline (speedup 1.0000x reference)
"""BinarySelfAttention Trainium2 kernel (8-core SPMD).

Strategy: shard (batch, head-group): core c -> batch c//4, heads 4*(c%4)..+3.
Each core computes ternary-projected QKV for its 4 heads, RoPE, causal
flash-style attention in S^T orientation (keys on partitions -> no transposes),
and a partial output projection against its Wo column slice. Host sums the 4
partials per batch.

All matmuls run in float32r (TF32-like, full PE rate at moving-dim >= 256).
Ternary weight signs {-1,0,1} are exact in f32r; ternary scales are folded
into the exp() scale (sq*sk/8) and the final output eviction (sv*so), both
passed as runtime data so the compiled program is input-independent.
"""
import numpy as np

import concourse.bass as bass
import concourse.mybir as mybir
import concourse.tile as tile
from concourse.bass_utils import run_bass_kernel_spmd
from concourse.tile_rust import add_dep_helper

F32 = mybir.dt.float32
F32R = mybir.dt.float32r

B, T, D, H = 2, 2048, 1024, 16
HD = 64            # head dim
HPC = 4            # heads per core
FPC = HPC * HD     # features per core (256)
NCORES = 8
KC = D // 128      # 8 contraction chunks for projections


def _split_excess_waits(nc, max_waits=1):
    """TRN2 ISA has one sem-wait slot per instruction and this walrus build
    rejects 3+; hoist excess waits onto preceding same-engine NOPs."""
    n = 0
    for f in nc.m.functions:
        for bb in f.blocks:
            new_insts = []
            for inst in bb.instructions:
                si = getattr(inst, 'sync_info', None)
                if si is not None and si.on_wait and len(si.on_wait) > max_waits:
                    waits = list(si.on_wait)
                    extra, keep = waits[:-max_waits], waits[-max_waits:]
                    for j, w in enumerate(extra):
                        new_insts.append(mybir.InstNoOp(
                            name=f"{inst.name}-wsplit{j}",
                            engine=inst.engine,
                            sync_info=mybir.SyncInfo(on_wait=[w], on_update=[]),
                            bass_nofuse=True,
                        ))
                        n += 1
                    inst.sync_info = mybir.SyncInfo(
                        on_wait=keep, on_update=si.on_update)
                new_insts.append(inst)
            bb.instructions[:] = new_insts
    return n


def _build():
    nc = bass.Bass("TRN2", target_bir_lowering=False, debug=False,
                   num_devices=NCORES)
    xt_d = nc.dram_tensor("xt", [D, T], F32R, kind="ExternalInput")
    wq_d = nc.dram_tensor("wqt", [D, FPC], F32R, kind="ExternalInput")
    wk_d = nc.dram_tensor("wkt", [D, FPC], F32R, kind="ExternalInput")
    wv_d = nc.dram_tensor("wvt", [D, FPC], F32R, kind="ExternalInput")
    wo_d = nc.dram_tensor("woc", [FPC, D], F32R, kind="ExternalInput")
    cos_d = nc.dram_tensor("cos2", [128, T], F32, kind="ExternalInput")
    sin_d = nc.dram_tensor("sins", [128, T], F32, kind="ExternalInput")
    msk_d = nc.dram_tensor("maskm", [128, 128], F32R, kind="ExternalInput")
    con_d = nc.dram_tensor("consts", [128, 2], F32, kind="ExternalInput")
    yp_d = nc.dram_tensor("yp", [T, D], F32, kind="ExternalOutput")
    rec_d = nc.dram_tensor("recd", [HPC, T], F32)  # internal scratch

    EXP = mybir.ActivationFunctionType.Exp
    CPY = mybir.ActivationFunctionType.Copy

    with tile.TileContext(nc) as tc:
        with tc.tile_pool(name="main", bufs=1) as mp:
            CON = mp.tile([128, 2], F32)
            MSK = mp.tile([128, 128], F32R)
            QT = [mp.tile([128, T], F32R, tag=f"qt{i}", name=f"qt{i}") for i in range(2)]
            KT = [mp.tile([128, T], F32R, tag=f"kt{i}", name=f"kt{i}") for i in range(2)]
            VA = mp.tile([128, 16, HPC * 65], F32R)
            ONES = mp.tile([128, 64], F32)

            nc.sync.dma_start(out=CON, in_=con_d[:, :])
            nc.sync.dma_start(out=MSK, in_=msk_d[:, :])
            nc.vector.memset(ONES, 1.0)
            ones_view = VA[:, :, :].rearrange(
                "p a (h e) -> p a h e", e=65)[:, :, :, 64:65].rearrange(
                "p a h e -> p (a h e)")
            nc.vector.tensor_copy(out=ones_view, in_=ONES[:, 0:64])

            # ---------------- Phase 1: projections + RoPE ----------------
            ptp_cm = tc.tile_pool(name="pt", bufs=7)
            ptp = ptp_cm.__enter__()
            with tc.tile_pool(name="p1", bufs=1) as p1, \
                 tc.tile_pool(name="wp", bufs=3) as wp:
                XT = p1.tile([128, KC, T], F32R)
                COS = p1.tile([128, T], F32)
                SIN = p1.tile([128, T], F32)

                _engs = [nc.sync, nc.scalar, nc.gpsimd]

                # kc-major interleave: weight chunk then its x chunks, so
                # the kc-streaming Q projection consumes data on arrival
                wts = {}
                for wname in ("q", "k", "v"):
                    wts[wname] = wp.tile([128, KC, FPC], F32R, tag="w",
                                         name=f"wt_{wname}")
                for kc in range(KC):
                    nc.sync.dma_start(
                        out=wts["q"][:, kc, :],
                        in_=wq_d[128 * kc:128 * kc + 128, :])
                    nc.gpsimd.dma_start(
                        out=wts["k"][:, kc, :],
                        in_=wk_d[128 * kc:128 * kc + 128, :])
                    for tch in range(4):
                        eng = nc.sync if tch % 2 == 0 else nc.scalar
                        eng.dma_start(
                            out=XT[:, kc, 512 * tch:512 * tch + 512],
                            in_=xt_d[128 * kc:128 * kc + 128,
                                     512 * tch:512 * tch + 512])
                for kc in range(KC):
                    nc.gpsimd.dma_start(
                        out=wts["v"][:, kc, :],
                        in_=wv_d[128 * kc:128 * kc + 128, :])
                nc.scalar.dma_start(out=COS, in_=cos_d[:, :])
                nc.scalar.dma_start(out=SIN, in_=sin_d[:, :])

                def proj_qk(wt, dest, evict_eng, psqk, pfx):
                    # kc-streaming: 8 persistent accumulators (8 PSUM banks)
                    accs = [psqk.tile([128, 512], F32, tag=f"pq{i}",
                                      name=f"{pfx}acc{i}") for i in range(8)]
                    for kc in range(KC):
                        for dt_i in range(2):
                            for tch in range(4):
                                nc.tensor.matmul(
                                    accs[4 * dt_i + tch],
                                    wt[:, kc, 128 * dt_i:128 * dt_i + 128],
                                    XT[:, kc, 512 * tch:512 * tch + 512],
                                    start=(kc == 0), stop=(kc == KC - 1))
                    for dt_i in range(2):
                        for tch in range(4):
                            eng = (nc.vector.tensor_copy if tch % 2 == 0
                                   else nc.scalar.copy)
                            eng(
                                out=dest[dt_i][:, 512 * tch:512 * tch + 512],
                                in_=accs[4 * dt_i + tch])

                def rope(dest, pfx):
                    # in-place rope on the f32r projection output
                    for dt_i in range(2):
                        dst = dest[dt_i]
                        rot = p1.tile([128, T], F32R, tag=f"rot{dt_i}",
                                      name=f"{pfx}rot{dt_i}")
                        for g in range(2):
                            b0 = 64 * g
                            nc.gpsimd.dma_start(out=rot[b0:b0 + 32, :],
                                                in_=dst[b0 + 32:b0 + 64, :])
                            nc.gpsimd.dma_start(out=rot[b0 + 32:b0 + 64, :],
                                                in_=dst[b0:b0 + 32, :])
                        nc.gpsimd.tensor_mul(rot, rot, SIN)
                        nc.vector.tensor_mul(dst, dst, COS)
                        nc.vector.tensor_add(dst, dst, rot)

                with tc.tile_pool(name="psqk", bufs=1,
                                  space="PSUM") as psqk:
                    proj_qk(wts["q"], QT, nc.vector.tensor_copy, psqk, "q")
                    proj_qk(wts["k"], KT, nc.vector.tensor_copy, psqk, "k")
                    rope(QT, "q")
                    rope(KT, "k")

                # attention pools open early: S/exp for (h0,qh0) is
                # prefetched before the V projection to hide V evictions
                pss_cm = tc.tile_pool(name="pss", bufs=2, space="PSUM")
                pss = pss_cm.__enter__()

                def s_exp_piece(h, qh, kc):
                    qt, kt = QT[h // 2], KT[h // 2]
                    r0 = 64 * (h % 2)
                    q0, q1 = 1024 * qh, 1024 * qh + 1024
                    qs = max(q0, 128 * kc)
                    cols = q1 - qs
                    sp = pss.tile([128, 1024], F32, tag="sp")
                    off = 0
                    while off < cols:
                        # a matmul must not cross a 512-f32 PSUM bank edge
                        cw = min(512 - (off % 512), cols - off)
                        nc.tensor.matmul(
                            sp[:, off:off + cw],
                            kt[r0:r0 + 64, 128 * kc:128 * kc + 128],
                            qt[r0:r0 + 64, qs + off:qs + off + cw],
                            start=True, stop=True)
                        off += cw
                    pt = ptp.tile([128, 1024], F32R, tag="pt")
                    nc.scalar.activation(
                        out=pt[:, 0:cols], in_=sp[:, 0:cols],
                        func=EXP, scale=CON[:, 0:1])
                    if 128 * kc >= q0:  # diagonal block leads piece
                        nc.vector.tensor_mul(
                            pt[:, 0:128], pt[:, 0:128], MSK)
                    return pt, qs, cols

                def pv_piece(yaug, h, qh, kc, pt, qs, cols):
                    q0 = 1024 * qh
                    off = 0
                    while off < cols:
                        # PV chunks aligned to 512-windows so each window's
                        # PSUM accumulation group is clean
                        cw = min(512 - ((qs + off) % 512), cols - off)
                        w = (qs + off) // 512
                        nc.tensor.matmul(
                            yaug[:, qs - q0 + off:qs - q0 + off + cw],
                            VA[:, kc, 65 * h:65 * h + 65],
                            pt[:, off:off + cw],
                            start=(kc == 0), stop=(kc == 4 * w + 3))
                        off += cw

                pre_pts = [s_exp_piece(0, 0, kc) for kc in range(4)]

                # V projection -> VA [keys, 4*(64+ones)]
                wtv = wts["v"]
                with tc.tile_pool(name="psv", bufs=4, space="PSUM") as psv:
                    for t16 in range(16):
                        acc = psv.tile([128, FPC], F32, tag="pv")
                        for kc in range(KC):
                            nc.tensor.matmul(
                                acc,
                                XT[:, kc, 128 * t16:128 * t16 + 128],
                                wtv[:, kc, :],
                                start=(kc == 0), stop=(kc == KC - 1))
                        veng = (nc.vector.tensor_copy if t16 % 2 == 0
                                else nc.scalar.copy)
                        veng(
                            out=VA[:, t16, :].rearrange(
                                "p (h e) -> p h e", e=65)[:, :, 0:64],
                            in_=acc.rearrange("p (h e) -> p h e", e=64))

            # ------- Phase 2: attention, q-halved for tail overlap -------
            atp_cm = tc.tile_pool(name="atp", bufs=1)
            atp = atp_cm.__enter__()
            AT = [atp.tile([128, T], F32R, tag=f"at{i}", name=f"at{i}")
                  for i in range(2)]
            WOC = atp.tile([128, 2, D], F32R)
            for ft in range(2):
                nc.sync.dma_start(out=WOC[:, ft, :],
                                  in_=wo_d[128 * ft:128 * ft + 128, :])
            with tc.tile_pool(name="p2", bufs=2) as p2, \
                 tc.tile_pool(name="rb", bufs=2) as rbp, \
                 tc.tile_pool(name="psy", bufs=2, space="PSUM") as psy:
                for h in (0, 1, 3, 2):
                    for qh in range(2):  # q half: [1024*qh, 1024*qh+1024)
                        q0, q1 = 1024 * qh, 1024 * qh + 1024
                        yaug = psy.tile([65, 1024], F32, tag="yaug")
                        for kc in range(8 * (qh + 1)):
                            if h == 0 and qh == 0 and kc < 4:
                                pt, qs, cols = pre_pts[kc]
                            else:
                                pt, qs, cols = s_exp_piece(h, qh, kc)
                            pv_piece(yaug, h, qh, kc, pt, qs, cols)
                        rec = p2.tile([1, 1024], F32, tag="rec")
                        nc.vector.reciprocal(out=rec, in_=yaug[64:65, :])
                        wr_i = nc.sync.dma_start(out=rec_d[h, q0:q1],
                                                 in_=rec)
                        rb = rbp.tile([64, 1024], F32, tag="rb")
                        rsrc = rec_d[h, q0:q1]
                        rd_i = nc.sync.dma_start(
                            out=rb,
                            in_=bass.AP(tensor=rsrc.tensor,
                                        offset=rsrc.offset,
                                        ap=[[0, 64]] + list(rsrc.ap)))
                        # Tile does not track DRAM scratch RAW deps
                        add_dep_helper(rd_i.ins, wr_i.ins, sync=True,
                                       reason="recd bounce RAW")
                        if h % 2 == 0:
                            nc.vector.tensor_mul(
                                AT[h // 2][0:64, q0:q1], yaug[0:64, :], rb)
                        else:
                            stg = p2.tile([64, 1024], F32R, tag="stg")
                            nc.vector.tensor_mul(stg, yaug[0:64, :], rb)
                            nc.sync.dma_start(
                                out=AT[h // 2][64:128, q0:q1], in_=stg)

            # ---------------- Phase 3: output projection ----------------
            with tc.tile_pool(name="p3", bufs=3) as p3, \
                 tc.tile_pool(name="pso", bufs=2, space="PSUM") as pso:
                for t16 in range(16):
                    yo = pso.tile([128, D], F32, tag="yo")
                    for half in range(2):
                        for ft in range(2):
                            nc.tensor.matmul(
                                yo[:, 512 * half:512 * half + 512],
                                AT[ft][:, 128 * t16:128 * t16 + 128],
                                WOC[:, ft, 512 * half:512 * half + 512],
                                start=(ft == 0), stop=(ft == 1))
                    ot = p3.tile([128, D], F32, tag="ot")
                    nc.scalar.activation(out=ot, in_=yo, func=CPY,
                                         scale=CON[:, 1:2])
                    nc.sync.dma_start(
                        out=yp_d[128 * t16:128 * t16 + 128, :], in_=ot)
            atp_cm.__exit__(None, None, None)
            ptp_cm.__exit__(None, None, None)
            pss_cm.__exit__(None, None, None)

    _split_excess_waits(nc)
    return nc


_NC = None
_LAST_INMAPS = None


def _get_nc():
    global _NC
    if _NC is None:
        _NC = _build()
    return _NC


def _ternary_signs(w):
    """Mirror reference ternary_weight: returns (signs in {-1,0,1}, scale)."""
    try:
        import jax
        import jax.numpy as jnp
        cpu = jax.devices("cpu")[0]
        with jax.default_device(cpu):
            wj = jnp.asarray(np.asarray(w, dtype=np.float32))
            scale = jnp.mean(jnp.abs(wj))
            signs = jnp.round(jnp.clip(wj / (scale + 1e-8), -1.0, 1.0))
            return np.asarray(signs, dtype=np.float32), float(scale)
    except Exception:
        w = np.asarray(w, dtype=np.float32)
        scale = np.float32(np.mean(np.abs(w)))
        signs = np.round(np.clip(w / (scale + np.float32(1e-8)), -1.0, 1.0))
        return signs.astype(np.float32), float(scale)


def _round12(a):
    """Round fp32 to 12 mantissa bits (representable in f32r)."""
    u = np.ascontiguousarray(a, dtype=np.float32).view(np.uint32)
    r = (u + np.uint32(1 << 10)) & np.uint32(0xFFFFF800)
    return r.view(np.float32)


def _rope_tables():
    inv = (1.0 / (10000.0 ** (np.arange(0, HD, 2, dtype=np.float32) / HD))
           ).astype(np.float32)                      # [32]
    t = np.arange(T, dtype=np.float32)
    fr = np.outer(t, inv).astype(np.float32)         # [T, 32]
    cos1 = np.cos(fr).astype(np.float32)             # [T, 32]
    sin1 = np.sin(fr).astype(np.float32)
    # rows: d in 0..63 (freq d%32), tiled for 2 heads -> 128 rows
    cosd = np.concatenate([cos1, cos1], axis=1).T    # [64, T]
    sind = np.concatenate([sin1, sin1], axis=1).T    # [64, T]
    sgn = np.ones((HD, 1), dtype=np.float32)
    sgn[:HD // 2] = -1.0
    cos2 = np.tile(cosd, (2, 1)).astype(np.float32)          # [128, T]
    sins = np.tile(sind * sgn, (2, 1)).astype(np.float32)    # [128, T]
    return cos2, sins


def kernel(x, Wq, Wk, Wv, Wo, mask):
    global _LAST_INMAPS
    x = np.asarray(x, dtype=np.float32)
    mask = np.asarray(mask)
    assert np.array_equal(
        np.asarray(mask[0, 0], dtype=np.int32),
        np.tril(np.ones((T, T), dtype=np.int32))), "non-causal mask"

    qs, sq = _ternary_signs(Wq)
    ks, sk = _ternary_signs(Wk)
    vs, sv = _ternary_signs(Wv)
    os_, so = _ternary_signs(Wo)
    cos2, sins = _rope_tables()
    mvals = np.triu(np.ones((128, 128), dtype=np.float32))  # valid: i <= j
    consts = np.zeros((128, 2), dtype=np.float32)
    consts[:, 0] = np.float32(sq) * np.float32(sk) * np.float32(0.125)
    consts[:, 1] = np.float32(sv) * np.float32(so)

    in_maps = []
    for c in range(NCORES):
        b, g = c // 4, c % 4
        fsl = slice(FPC * g, FPC * g + FPC)
        in_maps.append({
            "xt": _round12(x[b].T),
            "wqt": np.ascontiguousarray(qs[fsl].T),
            "wkt": np.ascontiguousarray(ks[fsl].T),
            "wvt": np.ascontiguousarray(vs[fsl].T),
            "woc": np.ascontiguousarray(os_[:, fsl].T),
            "cos2": cos2,
            "sins": sins,
            "maskm": mvals,
            "consts": consts,
        })
    _LAST_INMAPS = in_maps

    res = run_bass_kernel_spmd(_get_nc(), in_maps,
                               core_ids=list(range(NCORES)))
    out = np.zeros((B, T, D), dtype=np.float32)
    for b in range(B):
        acc = np.zeros((T, D), dtype=np.float32)
        for g in range(4):
            acc += res.results[4 * b + g]["yp"]
        out[b] = acc
    return out


def bench(trace=True):
    """Re-run last inputs with NTFF tracing; returns BassKernelResults."""
    assert _LAST_INMAPS is not None, "call kernel() first"
    return run_bass_kernel_spmd(_get_nc(), _LAST_INMAPS,
                                core_ids=list(range(NCORES)), trace=trace)



# revision 9
# speedup vs baseline: 1.1014x; 1.1014x over previous
"""BinarySelfAttention Trainium2 kernel (8-core SPMD), v2.

Strategy: shard (batch, head-group): core c -> batch c//4, heads 4*(c%4)..+3.
Each core computes ternary-projected QKV for its 4 heads (bf16 data path,
ternary signs exact in bf16), RoPE, causal attention in S^T orientation
(keys on partitions), a FLIPPED PV matmul (exp(S^T) chunks stationary, V
moving at 65 cols -> half the PE time of the 65-row orientation), per-
partition softmax normalization (no DRAM bounce), DMA-engine transposes of
the normalized y into [feature, T] layout, and a partial output projection
against its Wo column slice. Host sums the 4 bf16 partials per batch in f32.

Ternary scales are folded into the exp() scale (sq*sk/8) and the output
eviction (sv*so), passed as runtime data so the program is input-independent.
"""
import numpy as np
import ml_dtypes

import concourse.bass as bass
import concourse.mybir as mybir
import concourse.tile as tile
from concourse.bass_utils import run_bass_kernel_spmd

F32 = mybir.dt.float32
BF16 = mybir.dt.bfloat16
NPBF = ml_dtypes.bfloat16

B, T, D, H = 2, 2048, 1024, 16
HD = 64            # head dim
HPC = 4            # heads per core
FPC = HPC * HD     # features per core (256)
NCORES = 8
KC = D // 128      # 8 contraction chunks for projections


def _split_excess_waits(nc):
    """walrus wait-slot limits: 1 for most instructions, 0 for the DMA
    transpose, 1 for TensorScalarPtr; hoist excess onto same-queue NoOps."""
    LIMS = {"InstDmaTransposeAnt": 0, "InstTensorScalarPtr": 1}
    n = 0
    for f in nc.m.functions:
        for bb in f.blocks:
            new_insts = []
            for inst in bb.instructions:
                si = getattr(inst, 'sync_info', None)
                lim = LIMS.get(type(inst).__name__, 1)
                if si is not None and si.on_wait and len(si.on_wait) > lim:
                    waits = list(si.on_wait)
                    extra, keep = (waits, []) if lim == 0 else \
                        (waits[:-lim], waits[-lim:])
                    for j, w in enumerate(extra):
                        new_insts.append(mybir.InstNoOp(
                            name=f"{inst.name}-wsplit{j}",
                            engine=inst.engine,
                            sync_info=mybir.SyncInfo(on_wait=[w], on_update=[]),
                            bass_nofuse=True,
                        ))
                        n += 1
                    inst.sync_info = mybir.SyncInfo(
                        on_wait=keep, on_update=si.on_update)
                new_insts.append(inst)
            bb.instructions[:] = new_insts
    return n


def _build():
    nc = bass.Bass("TRN2", target_bir_lowering=False, debug=False,
                   num_devices=NCORES)
    xt_d = nc.dram_tensor("xt", [D, T], BF16, kind="ExternalInput")
    wq_d = nc.dram_tensor("wqt", [D, FPC], BF16, kind="ExternalInput")
    wk_d = nc.dram_tensor("wkt", [D, FPC], BF16, kind="ExternalInput")
    wv_d = nc.dram_tensor("wvt", [D, FPC], BF16, kind="ExternalInput")
    wo_d = nc.dram_tensor("woc", [FPC, D], BF16, kind="ExternalInput")
    cos_d = nc.dram_tensor("cos2", [128, T], BF16, kind="ExternalInput")
    sin_d = nc.dram_tensor("sins", [128, T], BF16, kind="ExternalInput")
    msk_d = nc.dram_tensor("maskm", [128, 128], BF16, kind="ExternalInput")
    con_d = nc.dram_tensor("consts", [128, 2], F32, kind="ExternalInput")
    yp_d = nc.dram_tensor("yp", [T, D], BF16, kind="ExternalOutput")

    EXP = mybir.ActivationFunctionType.Exp

    with tile.TileContext(nc) as tc:
        with tc.tile_pool(name="main", bufs=1) as mp:
            CON = mp.tile([128, 2], F32)
            MSK = mp.tile([128, 128], BF16)
            XT = mp.tile([128, KC, T], BF16)
            COS = mp.tile([128, T], BF16)
            SIN = mp.tile([128, T], BF16)
            QT = [mp.tile([128, T], BF16, name=f"qt{i}") for i in range(2)]
            KT = [mp.tile([128, T], BF16, name=f"kt{i}") for i in range(2)]
            VA = mp.tile([128, 16, HPC, 65], BF16)
            AT = [mp.tile([128, T], BF16, name=f"at{i}") for i in range(2)]
            WOC = mp.tile([128, 2, D], BF16)
            ONES = mp.tile([128, 64], BF16)

            nc.sync.dma_start(out=CON, in_=con_d[:, :])
            nc.sync.dma_start(out=MSK, in_=msk_d[:, :])
            nc.vector.memset(ONES, 1.0)
            ones_view = VA[:, :, :, 64:65].rearrange("p a h e -> p (a h e)")
            nc.vector.tensor_copy(out=ones_view, in_=ONES[:, 0:64])
            for ft in range(2):
                nc.sync.dma_start(out=WOC[:, ft, :],
                                  in_=wo_d[128 * ft:128 * ft + 128, :])

            # ---------------- Phase 1: projections + RoPE ----------------
            wp_cm = tc.tile_pool(name="wp", bufs=1)
            wp = wp_cm.__enter__()
            WQ = wp.tile([128, KC, FPC], BF16, name="wq")
            WK = wp.tile([128, KC, FPC], BF16, name="wk")
            WV = wp.tile([128, KC, FPC], BF16, name="wv")
            for kc in range(KC):
                nc.gpsimd.dma_start(out=WQ[:, kc, :],
                                    in_=wq_d[128 * kc:128 * kc + 128, :])
                nc.gpsimd.dma_start(out=WK[:, kc, :],
                                    in_=wk_d[128 * kc:128 * kc + 128, :])
                eng = nc.sync if kc % 2 == 0 else nc.scalar
                eng.dma_start(out=XT[:, kc, :],
                              in_=xt_d[128 * kc:128 * kc + 128, :])
            for kc in range(KC):
                nc.gpsimd.dma_start(out=WV[:, kc, :],
                                    in_=wv_d[128 * kc:128 * kc + 128, :])
            nc.scalar.dma_start(out=COS, in_=cos_d[:, :])
            nc.scalar.dma_start(out=SIN, in_=sin_d[:, :])

            def proj_qk(wt, dest, psqk, pfx):
                # kc-streaming: 8 persistent accumulators (8 PSUM banks)
                accs = [psqk.tile([128, 512], F32, tag=f"pq{i}",
                                  name=f"{pfx}acc{i}") for i in range(8)]
                for kc in range(KC):
                    for dt_i in range(2):
                        for tch in range(4):
                            nc.tensor.matmul(
                                accs[4 * dt_i + tch],
                                wt[:, kc, 128 * dt_i:128 * dt_i + 128],
                                XT[:, kc, 512 * tch:512 * tch + 512],
                                start=(kc == 0), stop=(kc == KC - 1))
                for dt_i in range(2):
                    for tch in range(4):
                        eng = (nc.vector.tensor_copy if tch % 2 == 0
                               else nc.scalar.copy)
                        eng(out=dest[dt_i][:, 512 * tch:512 * tch + 512],
                            in_=accs[4 * dt_i + tch])

            def rope(dest, rotp, pfx):
                for dt_i in range(2):
                    dst = dest[dt_i]
                    rot = rotp.tile([128, T], BF16, tag="rot",
                                    name=f"{pfx}rot{dt_i}")
                    for g in range(2):
                        b0 = 64 * g
                        nc.gpsimd.dma_start(out=rot[b0:b0 + 32, :],
                                            in_=dst[b0 + 32:b0 + 64, :])
                        nc.gpsimd.dma_start(out=rot[b0 + 32:b0 + 64, :],
                                            in_=dst[b0:b0 + 32, :])
                    nc.vector.tensor_mul(rot, rot, SIN)
                    nc.vector.tensor_mul(dst, dst, COS)
                    nc.vector.tensor_add(dst, dst, rot)

            with tc.tile_pool(name="rotp", bufs=2) as rotp:
                with tc.tile_pool(name="psqk", bufs=1, space="PSUM") as psqk:
                    proj_qk(WQ, QT, psqk, "q")
                    rope(QT, rotp, "q")
                    proj_qk(WK, KT, psqk, "k")
                    rope(KT, rotp, "k")

            # ------------- Phase 2: attention (+V proj, +O proj) ---------
            ptp_cm = tc.tile_pool(name="ptp", bufs=18)
            ptp = ptp_cm.__enter__()
            ybp_cm = tc.tile_pool(name="ybp", bufs=12)
            ybp = ybp_cm.__enter__()
            recp_cm = tc.tile_pool(name="recp", bufs=4)
            recp = recp_cm.__enter__()
            pss_cm = tc.tile_pool(name="pss", bufs=2, space="PSUM")
            pss = pss_cm.__enter__()
            psy_cm = tc.tile_pool(name="psy", bufs=2, space="PSUM")
            psy = psy_cm.__enter__()
            otp_cm = tc.tile_pool(name="otp", bufs=2)
            otp = otp_cm.__enter__()
            psv_cm = tc.tile_pool(name="psv", bufs=2, space="PSUM")
            psv = psv_cm.__enter__()

            ybufs = {}

            def v_chain(t16):
                acc = psv.tile([128, FPC], F32, tag="pv")
                for kc in range(KC):
                    nc.tensor.matmul(
                        acc, XT[:, kc, 128 * t16:128 * t16 + 128],
                        WV[:, kc, :], start=(kc == 0), stop=(kc == KC - 1))
                nc.vector.tensor_copy(
                    out=VA[:, t16, :, 0:64],
                    in_=acc.rearrange("p (h e) -> p h e", e=64))

            def s_piece(h, qh, kc):
                p, r0 = h // 2, 64 * (h % 2)
                q0 = 1024 * qh
                qs = max(q0, 128 * kc)
                cols = q0 + 1024 - qs
                sp = pss.tile([128, 1024], F32, tag="sp")
                off = 0
                while off < cols:
                    cw = min(512 - (off % 512), cols - off)
                    nc.tensor.matmul(
                        sp[:, off:off + cw],
                        KT[p][r0:r0 + 64, 128 * kc:128 * kc + 128],
                        QT[p][r0:r0 + 64, qs + off:qs + off + cw],
                        start=True, stop=True)
                    off += cw
                pt = ptp.tile([128, 1024], BF16, tag="pt")
                nc.scalar.activation(out=pt[:, 0:cols], in_=sp[:, 0:cols],
                                     func=EXP, scale=CON[:, 0:1])
                if 128 * kc >= q0:  # diagonal block leads the piece
                    nc.vector.tensor_mul(pt[:, 0:128], pt[:, 0:128], MSK)
                return pt, qs

            def attn(h, qh, with_v=False):
                q0 = 1024 * qh
                yqs = [psy.tile([128, 4, 128], F32, tag="yq",
                                name=f"yq_{h}_{qh}_{w}") for w in range(2)]
                nkc = 8 * (qh + 1)
                pts = {}

                def pv_chain(qb):
                    # sequential per-bank accumulation: chain qb fully
                    # start->stop before chain qb+1 opens (a start=True in a
                    # PSUM bank wipes still-open chains there)
                    for kc in range(8 * qh + qb + 1):
                        pt, qs = pts[kc]
                        off = 128 * qb + q0 - qs
                        nc.tensor.matmul(
                            yqs[qb // 4][:, qb % 4, 0:65],
                            pt[:, off:off + 128], VA[:, kc, h, :],
                            start=(kc == 0), stop=(kc == 8 * qh + qb))

                for kc in range(nkc):
                    pts[kc] = s_piece(h, qh, kc)
                    if with_v and kc < 8:
                        v_chain(8 * qh + kc)
                    if kc >= nkc - 8:
                        pv_chain(kc - (nkc - 8))
                # drain: normalize each 128-q block by its exp-sum
                p = h // 2
                for w in range(2):
                    yq = yqs[w]
                    rec = recp.tile([128, 4], F32, tag="rec")
                    nc.vector.reciprocal(
                        out=rec,
                        in_=yq[:, :, 64:65].rearrange("p a e -> p (a e)"))
                    for qb4 in range(4):
                        qbg = 8 * qh + 4 * w + qb4
                        if h % 2 == 0:
                            ybufs[(p, qbg)] = ybp.tile(
                                [128, 2, 64], BF16, tag="yb",
                                name=f"yb_{p}_{qbg}")
                        nc.vector.tensor_scalar_mul(
                            ybufs[(p, qbg)][:, h % 2, :],
                            yq[:, qb4, 0:64], rec[:, qb4:qb4 + 1])
                if h % 2 == 1:
                    for qb in range(8):
                        qbg = 8 * qh + qb
                        yb = ybufs.pop((p, qbg))
                        nc.sync.dma_start_transpose(
                            out=AT[p][:, 128 * qbg:128 * qbg + 128],
                            in_=yb[:, :, :].rearrange("p a e -> p (a e)"))

            def oproj(t16):
                ot = otp.tile([128, D], BF16, tag="ot")
                for half in range(2):
                    yo = pso.tile([128, 512], F32, tag="yo")
                    for ft in range(2):
                        nc.tensor.matmul(
                            yo, AT[ft][:, 128 * t16:128 * t16 + 128],
                            WOC[:, ft, 512 * half:512 * half + 512],
                            start=(ft == 0), stop=(ft == 1))
                    nc.vector.tensor_scalar_mul(
                        ot[:, 512 * half:512 * half + 512], yo, CON[:, 1:2])
                nc.sync.dma_start(out=yp_d[128 * t16:128 * t16 + 128, :],
                                  in_=ot)

            attn(0, 0, with_v=True)      # V chains t16 0..7 interleaved
            attn(1, 0)
            attn(2, 0)
            attn(3, 0)
            attn(0, 1, with_v=True)      # V chains t16 8..15 interleaved
            psv_cm.__exit__(None, None, None)
            pso_cm = tc.tile_pool(name="pso", bufs=2, space="PSUM")
            pso = pso_cm.__enter__()
            for t16 in range(0, 4):
                oproj(t16)
            attn(1, 1)
            for t16 in range(4, 8):
                oproj(t16)
            attn(2, 1)
            attn(3, 1)
            for t16 in range(8, 16):
                oproj(t16)

            pso_cm.__exit__(None, None, None)
            otp_cm.__exit__(None, None, None)
            psy_cm.__exit__(None, None, None)
            pss_cm.__exit__(None, None, None)
            recp_cm.__exit__(None, None, None)
            ybp_cm.__exit__(None, None, None)
            ptp_cm.__exit__(None, None, None)
            wp_cm.__exit__(None, None, None)

    _split_excess_waits(nc)
    return nc


_NC = None
_LAST_INMAPS = None


def _get_nc():
    global _NC
    if _NC is None:
        _NC = _build()
    return _NC


def _ternary_signs(w):
    """Mirror reference ternary_weight: returns (signs in {-1,0,1}, scale)."""
    try:
        import jax
        import jax.numpy as jnp
        cpu = jax.devices("cpu")[0]
        with jax.default_device(cpu):
            wj = jnp.asarray(np.asarray(w, dtype=np.float32))
            scale = jnp.mean(jnp.abs(wj))
            signs = jnp.round(jnp.clip(wj / (scale + 1e-8), -1.0, 1.0))
            return np.asarray(signs, dtype=np.float32), float(scale)
    except Exception:
        w = np.asarray(w, dtype=np.float32)
        scale = np.float32(np.mean(np.abs(w)))
        signs = np.round(np.clip(w / (scale + np.float32(1e-8)), -1.0, 1.0))
        return signs.astype(np.float32), float(scale)


def _rope_tables():
    inv = (1.0 / (10000.0 ** (np.arange(0, HD, 2, dtype=np.float32) / HD))
           ).astype(np.float32)                      # [32]
    t = np.arange(T, dtype=np.float32)
    fr = np.outer(t, inv).astype(np.float32)         # [T, 32]
    cos1 = np.cos(fr).astype(np.float32)
    sin1 = np.sin(fr).astype(np.float32)
    # rows: d in 0..63 (freq d%32), tiled for 2 heads -> 128 rows
    cosd = np.concatenate([cos1, cos1], axis=1).T    # [64, T]
    sind = np.concatenate([sin1, sin1], axis=1).T
    sgn = np.ones((HD, 1), dtype=np.float32)
    sgn[:HD // 2] = -1.0
    cos2 = np.tile(cosd, (2, 1)).astype(NPBF)                # [128, T]
    sins = np.tile(sind * sgn, (2, 1)).astype(NPBF)
    return cos2, sins


def kernel(x, Wq, Wk, Wv, Wo, mask):
    global _LAST_INMAPS
    x = np.asarray(x, dtype=np.float32)
    mask = np.asarray(mask)
    assert np.array_equal(
        np.asarray(mask[0, 0], dtype=np.int32),
        np.tril(np.ones((T, T), dtype=np.int32))), "non-causal mask"

    qs, sq = _ternary_signs(Wq)
    ks, sk = _ternary_signs(Wk)
    vs, sv = _ternary_signs(Wv)
    os_, so = _ternary_signs(Wo)
    cos2, sins = _rope_tables()
    mvals = np.triu(np.ones((128, 128), dtype=np.float32)).astype(NPBF)
    consts = np.zeros((128, 2), dtype=np.float32)
    consts[:, 0] = np.float32(sq) * np.float32(sk) * np.float32(0.125)
    consts[:, 1] = np.float32(sv) * np.float32(so)

    in_maps = []
    for c in range(NCORES):
        b, g = c // 4, c % 4
        fsl = slice(FPC * g, FPC * g + FPC)
        in_maps.append({
            "xt": np.ascontiguousarray(x[b].T).astype(NPBF),
            "wqt": np.ascontiguousarray(qs[fsl].T).astype(NPBF),
            "wkt": np.ascontiguousarray(ks[fsl].T).astype(NPBF),
            "wvt": np.ascontiguousarray(vs[fsl].T).astype(NPBF),
            "woc": np.ascontiguousarray(os_[:, fsl].T).astype(NPBF),
            "cos2": cos2,
            "sins": sins,
            "maskm": mvals,
            "consts": consts,
        })
    _LAST_INMAPS = in_maps

    res = run_bass_kernel_spmd(_get_nc(), in_maps,
                               core_ids=list(range(NCORES)))
    out = np.zeros((B, T, D), dtype=np.float32)
    for b in range(B):
        acc = np.zeros((T, D), dtype=np.float32)
        for g in range(4):
            acc += np.asarray(res.results[4 * b + g]["yp"],
                              dtype=np.float32)
        out[b] = acc
    return out


def bench(trace=True):
    """Re-run last inputs with NTFF tracing; returns BassKernelResults."""
    assert _LAST_INMAPS is not None, "call kernel() first"
    return run_bass_kernel_spmd(_get_nc(), _LAST_INMAPS,
                                core_ids=list(range(NCORES)), trace=trace)


# revision 13
# speedup vs baseline: 1.1896x; 1.0802x over previous
"""BinarySelfAttention Trainium2 kernel (8-core SPMD), v2.

Strategy: shard (batch, head-group): core c -> batch c//4, heads 4*(c%4)..+3.
Each core computes ternary-projected QKV for its 4 heads (bf16 data path,
ternary signs exact in bf16), RoPE, causal attention in S^T orientation
(keys on partitions), a FLIPPED PV matmul (exp(S^T) chunks stationary, V
moving at 65 cols -> half the PE time of the 65-row orientation), per-
partition softmax normalization (no DRAM bounce), DMA-engine transposes of
the normalized y into [feature, T] layout, and a partial output projection
against its Wo column slice. Host sums the 4 bf16 partials per batch in f32.

Ternary scales are folded into the exp() scale (sq*sk/8) and the output
eviction (sv*so), passed as runtime data so the program is input-independent.
"""
import numpy as np
import ml_dtypes

import concourse.bass as bass
import concourse.mybir as mybir
import concourse.tile as tile
from concourse.bass_utils import run_bass_kernel_spmd

F32 = mybir.dt.float32
BF16 = mybir.dt.bfloat16
NPBF = ml_dtypes.bfloat16

B, T, D, H = 2, 2048, 1024, 16
HD = 64            # head dim
HPC = 4            # heads per core
FPC = HPC * HD     # features per core (256)
NCORES = 8
KC = D // 128      # 8 contraction chunks for projections


def _split_excess_waits(nc):
    """walrus wait-slot limits: 1 for most instructions, 0 for the DMA
    transpose, 1 for TensorScalarPtr; hoist excess onto same-queue NoOps."""
    LIMS = {"InstDmaTransposeAnt": 0, "InstTensorScalarPtr": 1}
    n = 0
    for f in nc.m.functions:
        for bb in f.blocks:
            new_insts = []
            for inst in bb.instructions:
                si = getattr(inst, 'sync_info', None)
                lim = LIMS.get(type(inst).__name__, 1)
                if si is not None and si.on_wait and len(si.on_wait) > lim:
                    waits = list(si.on_wait)
                    extra, keep = (waits, []) if lim == 0 else \
                        (waits[:-lim], waits[-lim:])
                    for j, w in enumerate(extra):
                        new_insts.append(mybir.InstNoOp(
                            name=f"{inst.name}-wsplit{j}",
                            engine=inst.engine,
                            sync_info=mybir.SyncInfo(on_wait=[w], on_update=[]),
                            bass_nofuse=True,
                        ))
                        n += 1
                    inst.sync_info = mybir.SyncInfo(
                        on_wait=keep, on_update=si.on_update)
                new_insts.append(inst)
            bb.instructions[:] = new_insts
    return n


def _build():
    nc = bass.Bass("TRN2", target_bir_lowering=False, debug=False,
                   num_devices=NCORES)
    xt_d = nc.dram_tensor("xt", [D, T], BF16, kind="ExternalInput")
    wq_d = nc.dram_tensor("wqt", [D, FPC], BF16, kind="ExternalInput")
    wk_d = nc.dram_tensor("wkt", [D, FPC], BF16, kind="ExternalInput")
    wv_d = nc.dram_tensor("wvt", [D, FPC], BF16, kind="ExternalInput")
    wo_d = nc.dram_tensor("woc", [FPC, D], BF16, kind="ExternalInput")
    cos_d = nc.dram_tensor("cos2", [128, T], BF16, kind="ExternalInput")
    sin_d = nc.dram_tensor("sins", [128, T], BF16, kind="ExternalInput")
    msk_d = nc.dram_tensor("maskm", [128, 128], BF16, kind="ExternalInput")
    con_d = nc.dram_tensor("consts", [128, 2], F32, kind="ExternalInput")
    yp_d = nc.dram_tensor("yp", [T, D], BF16, kind="ExternalOutput")

    EXP = mybir.ActivationFunctionType.Exp

    with tile.TileContext(nc) as tc:
        with tc.tile_pool(name="main", bufs=1) as mp:
            CON = mp.tile([128, 2], F32)
            MSK = mp.tile([128, 128], BF16)
            XT = mp.tile([128, KC, T], BF16)
            COS = mp.tile([128, T], BF16)
            SIN = mp.tile([128, T], BF16)
            QT = [mp.tile([128, T], BF16, name=f"qt{i}") for i in range(2)]
            KT = [mp.tile([128, T], BF16, name=f"kt{i}") for i in range(2)]
            VA = mp.tile([128, 16, HPC, 65], BF16)
            AT = [mp.tile([128, T], BF16, name=f"at{i}") for i in range(2)]
            WOC = mp.tile([128, 2, D], BF16)
            ONES = mp.tile([128, 64], BF16)

            # DMA issue order drives the serial DMA device: x chunks first
            # (kc-streaming projections), each ternary weight as ONE batched
            # SWDGE transfer, Wo/consts deferred (needed late).
            WQ = mp.tile([128, KC, FPC], BF16, name="wq")
            WK = mp.tile([128, KC, FPC], BF16, name="wk")
            WV = mp.tile([128, KC, FPC], BF16, name="wv")
            for kc in range(KC):
                eng = nc.sync if kc % 2 == 0 else nc.scalar
                eng.dma_start(out=XT[:, kc, :],
                              in_=xt_d[128 * kc:128 * kc + 128, :])
            nc.gpsimd.dma_start(
                out=WQ[:, :, :],
                in_=wq_d.rearrange("(kc p) f -> p kc f", p=128))
            nc.gpsimd.dma_start(
                out=WK[:, :, :],
                in_=wk_d.rearrange("(kc p) f -> p kc f", p=128))
            nc.gpsimd.dma_start(
                out=WV[:, :, :],
                in_=wv_d.rearrange("(kc p) f -> p kc f", p=128))
            nc.scalar.dma_start(out=COS, in_=cos_d[:, :])
            nc.scalar.dma_start(out=SIN, in_=sin_d[:, :])
            nc.sync.dma_start(out=CON, in_=con_d[:, :])
            nc.sync.dma_start(out=MSK, in_=msk_d[:, :])
            nc.vector.memset(ONES, 1.0)
            ones_view = VA[:, :, :, 64:65].rearrange("p a h e -> p (a h e)")
            nc.vector.tensor_copy(out=ones_view, in_=ONES[:, 0:64])
            for ft in range(2):
                nc.scalar.dma_start(out=WOC[:, ft, :],
                                    in_=wo_d[128 * ft:128 * ft + 128, :])

            def proj_qk(wt, dest, psqk, pfx):
                # kc-streaming: 8 persistent accumulators (8 PSUM banks)
                accs = [psqk.tile([128, 512], F32, tag=f"pq{i}",
                                  name=f"{pfx}acc{i}") for i in range(8)]
                for kc in range(KC):
                    for dt_i in range(2):
                        for tch in range(4):
                            nc.tensor.matmul(
                                accs[4 * dt_i + tch],
                                wt[:, kc, 128 * dt_i:128 * dt_i + 128],
                                XT[:, kc, 512 * tch:512 * tch + 512],
                                start=(kc == 0), stop=(kc == KC - 1))
                for dt_i in range(2):
                    for tch in range(4):
                        eng = (nc.vector.tensor_copy if tch % 2 == 0
                               else nc.scalar.copy)
                        eng(out=dest[dt_i][:, 512 * tch:512 * tch + 512],
                            in_=accs[4 * dt_i + tch])

            def rope(dest, rotp, pfx):
                for dt_i in range(2):
                    dst = dest[dt_i]
                    rot = rotp.tile([128, T], BF16, tag="rot",
                                    name=f"{pfx}rot{dt_i}")
                    for g in range(2):
                        b0 = 64 * g
                        nc.gpsimd.dma_start(out=rot[b0:b0 + 32, :],
                                            in_=dst[b0 + 32:b0 + 64, :])
                        nc.gpsimd.dma_start(out=rot[b0 + 32:b0 + 64, :],
                                            in_=dst[b0:b0 + 32, :])
                    nc.vector.tensor_mul(rot, rot, SIN)
                    nc.vector.tensor_mul(dst, dst, COS)
                    nc.vector.tensor_add(dst, dst, rot)

            with tc.tile_pool(name="rotp", bufs=2) as rotp:
                with tc.tile_pool(name="psqk", bufs=1, space="PSUM") as psqk:
                    proj_qk(WQ, QT, psqk, "q")
                    rope(QT, rotp, "q")
                    proj_qk(WK, KT, psqk, "k")
                    rope(KT, rotp, "k")
            del WQ, WK

            # ------------- Phase 2: attention (+V proj, +O proj) ---------
            # Software-pipelined heads: head h's PV chains interleave with
            # head h+1's S/exp pieces so the Act engine (exp) stays fed by
            # the in-order PE stream; V chains and O-proj chunks fill
            # leftover PE slack.
            ptp_cm = tc.tile_pool(name="ptp", bufs=36)
            ptp = ptp_cm.__enter__()
            ybp_cm = tc.tile_pool(name="ybp", bufs=12)
            ybp = ybp_cm.__enter__()
            recp_cm = tc.tile_pool(name="recp", bufs=4)
            recp = recp_cm.__enter__()
            pss_cm = tc.tile_pool(name="pss", bufs=2, space="PSUM")
            pss = pss_cm.__enter__()
            psy_cm = tc.tile_pool(name="psy", bufs=2, space="PSUM")
            psy = psy_cm.__enter__()
            otp_cm = tc.tile_pool(name="otp", bufs=2)
            otp = otp_cm.__enter__()
            psv_cm = tc.tile_pool(name="psv", bufs=2, space="PSUM")
            psv = psv_cm.__enter__()

            ybufs = {}
            pso = None

            def v_chain(t16):
                acc = psv.tile([128, FPC], F32, tag="pv")
                for kc in range(KC):
                    nc.tensor.matmul(
                        acc, XT[:, kc, 128 * t16:128 * t16 + 128],
                        WV[:, kc, :], start=(kc == 0), stop=(kc == KC - 1))
                nc.vector.tensor_copy(
                    out=VA[:, t16, :, 0:64],
                    in_=acc.rearrange("p (h e) -> p h e", e=64))

            def s_pieces(h, qh, with_v=False):
                """Generator: one S/exp piece per next() (+ V chain)."""
                p, r0 = h // 2, 64 * (h % 2)
                q0 = 1024 * qh
                pts = {}
                for kc in range(8 * (qh + 1)):
                    qs = max(q0, 128 * kc)
                    cols = q0 + 1024 - qs
                    sp = pss.tile([128, 1024], F32, tag="sp")
                    off = 0
                    while off < cols:
                        cw = min(512 - (off % 512), cols - off)
                        nc.tensor.matmul(
                            sp[:, off:off + cw],
                            KT[p][r0:r0 + 64, 128 * kc:128 * kc + 128],
                            QT[p][r0:r0 + 64, qs + off:qs + off + cw],
                            start=True, stop=True)
                        off += cw
                    pt = ptp.tile([128, 1024], BF16, tag="pt")
                    nc.scalar.activation(out=pt[:, 0:cols], in_=sp[:, 0:cols],
                                         func=EXP, scale=CON[:, 0:1])
                    if 128 * kc >= q0:  # diagonal block leads the piece
                        nc.vector.tensor_mul(pt[:, 0:128], pt[:, 0:128], MSK)
                    pts[kc] = (pt, qs)
                    if with_v and kc < 8:
                        v_chain(8 * qh + kc)
                    yield pts

            def pv_chains(h, qh, pts):
                """Generator: one PV accumulation chain (q-block) per next().
                Chains are sequential per PSUM bank: a start=True in a bank
                wipes still-open chains there, so chain qb fully closes
                before chain qb+1 opens."""
                q0 = 1024 * qh
                yqs = [psy.tile([128, 4, 128], F32, tag="yq",
                                name=f"yq_{h}_{qh}_{w}") for w in range(2)]
                for qb in range(8):
                    for kc in range(8 * qh + qb + 1):
                        pt, qs = pts[kc]
                        off = 128 * qb + q0 - qs
                        nc.tensor.matmul(
                            yqs[qb // 4][:, qb % 4, 0:65],
                            pt[:, off:off + 128], VA[:, kc, h, :],
                            start=(kc == 0), stop=(kc == 8 * qh + qb))
                    yield
                # drain: normalize each 128-q block by its exp-sum
                p = h // 2
                for w in range(2):
                    yq = yqs[w]
                    rec = recp.tile([128, 4], F32, tag="rec")
                    nc.vector.reciprocal(
                        out=rec,
                        in_=yq[:, :, 64:65].rearrange("p a e -> p (a e)"))
                    for qb4 in range(4):
                        qbg = 8 * qh + 4 * w + qb4
                        if h % 2 == 0:
                            ybufs[(p, qbg)] = ybp.tile(
                                [128, 2, 64], BF16, tag="yb",
                                name=f"yb_{p}_{qbg}")
                        nc.vector.tensor_scalar_mul(
                            ybufs[(p, qbg)][:, h % 2, :],
                            yq[:, qb4, 0:64], rec[:, qb4:qb4 + 1])
                if h % 2 == 1:
                    for qb in range(8):
                        qbg = 8 * qh + qb
                        yb = ybufs.pop((p, qbg))
                        nc.sync.dma_start_transpose(
                            out=AT[p][:, 128 * qbg:128 * qbg + 128],
                            in_=yb[:, :, :].rearrange("p a e -> p (a e)"))
                yield

            def oproj(t16):
                ot = otp.tile([128, D], BF16, tag="ot")
                for half in range(2):
                    yo = pso.tile([128, 512], F32, tag="yo")
                    for ft in range(2):
                        nc.tensor.matmul(
                            yo, AT[ft][:, 128 * t16:128 * t16 + 128],
                            WOC[:, ft, 512 * half:512 * half + 512],
                            start=(ft == 0), stop=(ft == 1))
                    nc.vector.tensor_scalar_mul(
                        ot[:, 512 * half:512 * half + 512], yo, CON[:, 1:2])
                nc.sync.dma_start(out=yp_d[128 * t16:128 * t16 + 128, :],
                                  in_=ot)

            def drive(chain_gen, piece_gen, opro=(), ratio=1):
                """Round-robin: PV chains of head h with S pieces of head
                h+1 (and O-proj chunks) until all exhausted."""
                opro = list(opro)
                last = None
                c_done = chain_gen is None
                p_done = piece_gen is None
                while not (c_done and p_done and not opro):
                    if not c_done:
                        try:
                            next(chain_gen)
                        except StopIteration:
                            c_done = True
                    if opro:
                        oproj(opro.pop(0))
                    if not p_done:
                        for _ in range(ratio):
                            try:
                                last = next(piece_gen)
                            except StopIteration:
                                p_done = True
                                break
                return last

            # ---- pipelined schedule ----
            HEADS = [(h, qh) for qh in (0, 1) for h in range(4)]
            chain_gen = None
            pso_cm = None
            for h, qh in HEADS:
                if (h, qh) == (1, 1):
                    # all V chains emitted inside (0,1)'s pieces: psv -> pso
                    psv_cm.__exit__(None, None, None)
                    pso_cm = tc.tile_pool(name="pso", bufs=2, space="PSUM")
                    pso = pso_cm.__enter__()
                piece_gen = s_pieces(h, qh, with_v=(h == 0))
                opro = ()
                if (h, qh) == (2, 1):
                    opro = range(0, 4)
                elif (h, qh) == (3, 1):
                    opro = range(4, 8)
                pts = drive(chain_gen, piece_gen, opro, ratio=qh + 1)
                chain_gen = pv_chains(h, qh, pts)
            drive(chain_gen, None)
            for t16 in range(8, 16):
                oproj(t16)

            pso_cm.__exit__(None, None, None)
            otp_cm.__exit__(None, None, None)
            psy_cm.__exit__(None, None, None)
            pss_cm.__exit__(None, None, None)
            recp_cm.__exit__(None, None, None)
            ybp_cm.__exit__(None, None, None)
            ptp_cm.__exit__(None, None, None)

    _split_excess_waits(nc)
    return nc


_NC = None
_LAST_INMAPS = None


def _get_nc():
    global _NC
    if _NC is None:
        _NC = _build()
    return _NC


def _ternary_signs(w):
    """Mirror reference ternary_weight: returns (signs in {-1,0,1}, scale)."""
    try:
        import jax
        import jax.numpy as jnp
        cpu = jax.devices("cpu")[0]
        with jax.default_device(cpu):
            wj = jnp.asarray(np.asarray(w, dtype=np.float32))
            scale = jnp.mean(jnp.abs(wj))
            signs = jnp.round(jnp.clip(wj / (scale + 1e-8), -1.0, 1.0))
            return np.asarray(signs, dtype=np.float32), float(scale)
    except Exception:
        w = np.asarray(w, dtype=np.float32)
        scale = np.float32(np.mean(np.abs(w)))
        signs = np.round(np.clip(w / (scale + np.float32(1e-8)), -1.0, 1.0))
        return signs.astype(np.float32), float(scale)


def _rope_tables():
    inv = (1.0 / (10000.0 ** (np.arange(0, HD, 2, dtype=np.float32) / HD))
           ).astype(np.float32)                      # [32]
    t = np.arange(T, dtype=np.float32)
    fr = np.outer(t, inv).astype(np.float32)         # [T, 32]
    cos1 = np.cos(fr).astype(np.float32)
    sin1 = np.sin(fr).astype(np.float32)
    # rows: d in 0..63 (freq d%32), tiled for 2 heads -> 128 rows
    cosd = np.concatenate([cos1, cos1], axis=1).T    # [64, T]
    sind = np.concatenate([sin1, sin1], axis=1).T
    sgn = np.ones((HD, 1), dtype=np.float32)
    sgn[:HD // 2] = -1.0
    cos2 = np.tile(cosd, (2, 1)).astype(NPBF)                # [128, T]
    sins = np.tile(sind * sgn, (2, 1)).astype(NPBF)
    return cos2, sins


def kernel(x, Wq, Wk, Wv, Wo, mask):
    global _LAST_INMAPS
    x = np.asarray(x, dtype=np.float32)
    mask = np.asarray(mask)
    assert np.array_equal(
        np.asarray(mask[0, 0], dtype=np.int32),
        np.tril(np.ones((T, T), dtype=np.int32))), "non-causal mask"

    qs, sq = _ternary_signs(Wq)
    ks, sk = _ternary_signs(Wk)
    vs, sv = _ternary_signs(Wv)
    os_, so = _ternary_signs(Wo)
    cos2, sins = _rope_tables()
    mvals = np.triu(np.ones((128, 128), dtype=np.float32)).astype(NPBF)
    consts = np.zeros((128, 2), dtype=np.float32)
    consts[:, 0] = np.float32(sq) * np.float32(sk) * np.float32(0.125)
    consts[:, 1] = np.float32(sv) * np.float32(so)

    in_maps = []
    for c in range(NCORES):
        b, g = c // 4, c % 4
        fsl = slice(FPC * g, FPC * g + FPC)
        in_maps.append({
            "xt": np.ascontiguousarray(x[b].T).astype(NPBF),
            "wqt": np.ascontiguousarray(qs[fsl].T).astype(NPBF),
            "wkt": np.ascontiguousarray(ks[fsl].T).astype(NPBF),
            "wvt": np.ascontiguousarray(vs[fsl].T).astype(NPBF),
            "woc": np.ascontiguousarray(os_[:, fsl].T).astype(NPBF),
            "cos2": cos2,
            "sins": sins,
            "maskm": mvals,
            "consts": consts,
        })
    _LAST_INMAPS = in_maps

    res = run_bass_kernel_spmd(_get_nc(), in_maps,
                               core_ids=list(range(NCORES)))
    out = np.zeros((B, T, D), dtype=np.float32)
    for b in range(B):
        acc = np.zeros((T, D), dtype=np.float32)
        for g in range(4):
            acc += np.asarray(res.results[4 * b + g]["yp"],
                              dtype=np.float32)
        out[b] = acc
    return out


def bench(trace=True):
    """Re-run last inputs with NTFF tracing; returns BassKernelResults."""
    assert _LAST_INMAPS is not None, "call kernel() first"
    return run_bass_kernel_spmd(_get_nc(), _LAST_INMAPS,
                                core_ids=list(range(NCORES)), trace=trace)


# revision 20
# speedup vs baseline: 1.2970x; 1.0903x over previous
"""BinarySelfAttention Trainium2 kernel (8-core SPMD), v2.

Strategy: shard (batch, head-group): core c -> batch c//4, heads 4*(c%4)..+3.
Each core computes ternary-projected QKV for its 4 heads (bf16 data path,
ternary signs exact in bf16), RoPE, causal attention in S^T orientation
(keys on partitions), a FLIPPED PV matmul (exp(S^T) chunks stationary, V
moving at 65 cols -> half the PE time of the 65-row orientation), per-
partition softmax normalization (no DRAM bounce), DMA-engine transposes of
the normalized y into [feature, T] layout, and a partial output projection
against its Wo column slice. Host sums the 4 bf16 partials per batch in f32.

Ternary scales are folded into the exp() scale (sq*sk/8) and the output
eviction (sv*so), passed as runtime data so the program is input-independent.
"""
import numpy as np
import ml_dtypes

import concourse.bass as bass
import concourse.mybir as mybir
import concourse.tile as tile
from concourse.bass_utils import run_bass_kernel_spmd

F32 = mybir.dt.float32
BF16 = mybir.dt.bfloat16
NPBF = ml_dtypes.bfloat16

B, T, D, H = 2, 2048, 1024, 16
HD = 64            # head dim
HPC = 4            # heads per core
FPC = HPC * HD     # features per core (256)
NCORES = 8
KC = D // 128      # 8 contraction chunks for projections


def _split_excess_waits(nc):
    """walrus wait-slot limits: 1 for most instructions, 0 for the DMA
    transpose, 1 for TensorScalarPtr; hoist excess onto same-queue NoOps."""
    LIMS = {"InstDmaTransposeAnt": 0, "InstTensorScalarPtr": 1}
    n = 0
    for f in nc.m.functions:
        for bb in f.blocks:
            new_insts = []
            for inst in bb.instructions:
                si = getattr(inst, 'sync_info', None)
                lim = LIMS.get(type(inst).__name__, 1)
                if si is not None and si.on_wait and len(si.on_wait) > lim:
                    waits = list(si.on_wait)
                    extra, keep = (waits, []) if lim == 0 else \
                        (waits[:-lim], waits[-lim:])
                    for j, w in enumerate(extra):
                        new_insts.append(mybir.InstNoOp(
                            name=f"{inst.name}-wsplit{j}",
                            engine=inst.engine,
                            sync_info=mybir.SyncInfo(on_wait=[w], on_update=[]),
                            bass_nofuse=True,
                        ))
                        n += 1
                    inst.sync_info = mybir.SyncInfo(
                        on_wait=keep, on_update=si.on_update)
                new_insts.append(inst)
            bb.instructions[:] = new_insts
    return n


def _build():
    nc = bass.Bass("TRN2", target_bir_lowering=False, debug=False,
                   num_devices=NCORES)
    xt_d = nc.dram_tensor("xt", [D, T], BF16, kind="ExternalInput")
    wq_d = nc.dram_tensor("wqt", [D, FPC], BF16, kind="ExternalInput")
    wk_d = nc.dram_tensor("wkt", [D, FPC], BF16, kind="ExternalInput")
    wv_d = nc.dram_tensor("wvt", [D, FPC], BF16, kind="ExternalInput")
    wo_d = nc.dram_tensor("woc", [FPC, D], BF16, kind="ExternalInput")
    cos_d = nc.dram_tensor("cos2", [128, T], BF16, kind="ExternalInput")
    sin_d = nc.dram_tensor("sins", [128, T], BF16, kind="ExternalInput")
    msk_d = nc.dram_tensor("maskm", [128, 128], BF16, kind="ExternalInput")
    con_d = nc.dram_tensor("consts", [128, 2], F32, kind="ExternalInput")
    yp_d = nc.dram_tensor("yp", [T, D], BF16, kind="ExternalOutput")

    EXP = mybir.ActivationFunctionType.Exp
    CPY = mybir.ActivationFunctionType.Copy

    with tile.TileContext(nc) as tc:
        with tc.tile_pool(name="main", bufs=1) as mp:
            CON = mp.tile([128, 2], F32)
            MSK = mp.tile([128, 128], BF16)
            XT = mp.tile([128, KC, T], BF16)
            COS = mp.tile([128, T], BF16)
            SIN = mp.tile([128, T], BF16)
            QT = [mp.tile([128, T], BF16, name=f"qt{i}") for i in range(2)]
            KT = [mp.tile([128, T], BF16, name=f"kt{i}") for i in range(2)]
            VA = mp.tile([128, 16, HPC, 65], BF16)
            AT = [mp.tile([128, T], BF16, name=f"at{i}") for i in range(2)]
            WOC = mp.tile([128, 2, D], BF16)
            ONES = mp.tile([128, 64], BF16)

            # DMA issue order drives the serial DMA device: x chunks first
            # (kc-streaming projections), each ternary weight as ONE batched
            # SWDGE transfer, Wo/consts deferred (needed late).
            WQ = mp.tile([128, KC, FPC], BF16, name="wq")
            WK = mp.tile([128, KC, FPC], BF16, name="wk")
            WV = mp.tile([128, KC, FPC], BF16, name="wv")
            nc.sync.dma_start(out=XT[:, 0, 0:1024],
                              in_=xt_d[0:128, 0:1024])
            nc.gpsimd.dma_start(
                out=WQ[:, 0:4, :],
                in_=wq_d[0:512, :].rearrange("(kc p) f -> p kc f", p=128))
            nc.sync.dma_start(out=XT[:, 0, 1024:2048],
                              in_=xt_d[0:128, 1024:2048])
            nc.gpsimd.dma_start(
                out=WQ[:, 4:8, :],
                in_=wq_d[512:1024, :].rearrange("(kc p) f -> p kc f", p=128))
            for kc in range(1, KC):
                eng = nc.sync if kc % 2 == 0 else nc.scalar
                eng.dma_start(out=XT[:, kc, :],
                              in_=xt_d[128 * kc:128 * kc + 128, :])
            nc.gpsimd.dma_start(
                out=WK[:, :, :],
                in_=wk_d.rearrange("(kc p) f -> p kc f", p=128))
            nc.gpsimd.dma_start(
                out=WV[:, :, :],
                in_=wv_d.rearrange("(kc p) f -> p kc f", p=128))
            nc.scalar.dma_start(out=COS, in_=cos_d[:, :])
            nc.scalar.dma_start(out=SIN, in_=sin_d[:, :])
            nc.sync.dma_start(out=CON, in_=con_d[:, :])
            nc.sync.dma_start(out=MSK, in_=msk_d[:, :])
            nc.vector.memset(ONES, 1.0)
            ones_view = VA[:, :, :, 64:65].rearrange("p a h e -> p (a h e)")
            nc.vector.tensor_copy(out=ones_view, in_=ONES[:, 0:64])
            for ft in range(2):
                nc.scalar.dma_start(out=WOC[:, ft, :],
                                    in_=wo_d[128 * ft:128 * ft + 128, :])

            def proj_qk(wt, dest, psqk, pfx):
                # kc-streaming: 8 persistent accumulators (8 PSUM banks);
                # evictions on Act (idle in phase 1) to keep DVE free for rope
                accs = [psqk.tile([128, 512], F32, tag=f"pq{i}",
                                  name=f"{pfx}acc{i}") for i in range(8)]
                for kc in range(KC):
                    for dt_i in range(2):
                        for tch in range(4):
                            nc.tensor.matmul(
                                accs[4 * dt_i + tch],
                                wt[:, kc, 128 * dt_i:128 * dt_i + 128],
                                XT[:, kc, 512 * tch:512 * tch + 512],
                                start=(kc == 0), stop=(kc == KC - 1))
                for dt_i in range(2):
                    for tch in range(4):
                        eng = (nc.scalar.copy if tch % 2 == 0
                               else nc.vector.tensor_copy)
                        eng(out=dest[dt_i][:, 512 * tch:512 * tch + 512],
                            in_=accs[4 * dt_i + tch])

            def rope(dest, rotp, pfx, dt_i):
                dst = dest[dt_i]
                rot = rotp.tile([128, T], BF16, tag="rot",
                                name=f"{pfx}rot{dt_i}")
                for g in range(2):
                    b0 = 64 * g
                    nc.gpsimd.dma_start(out=rot[b0:b0 + 32, :],
                                        in_=dst[b0 + 32:b0 + 64, :])
                    nc.gpsimd.dma_start(out=rot[b0 + 32:b0 + 64, :],
                                        in_=dst[b0:b0 + 32, :])
                nc.vector.tensor_mul(rot, rot, SIN)
                nc.vector.tensor_mul(dst, dst, COS)
                nc.vector.tensor_add(dst, dst, rot)

            with tc.tile_pool(name="rotp", bufs=2) as rotp:
                with tc.tile_pool(name="psqk", bufs=1, space="PSUM") as psqk:
                    proj_qk(WQ, QT, psqk, "q")
                    rope(QT, rotp, "q", 0)     # pair 0 first: attention on
                    proj_qk(WK, KT, psqk, "k")  # heads 0,1 starts sooner
                    rope(KT, rotp, "k", 0)
                    rope(QT, rotp, "q", 1)
                    rope(KT, rotp, "k", 1)

            # ------------- Phase 2: attention (+V proj, +O proj) ---------
            # Software-pipelined heads: head h's PV chains interleave with
            # head h+1's S/exp pieces so the Act engine (exp) stays fed by
            # the in-order PE stream; V chains and O-proj chunks fill
            # leftover PE slack.
            ptp_cm = tc.tile_pool(name="ptp", bufs=36)
            ptp = ptp_cm.__enter__()
            ybp_cm = tc.tile_pool(name="ybp", bufs=12)
            ybp = ybp_cm.__enter__()
            recp_cm = tc.tile_pool(name="recp", bufs=4)
            recp = recp_cm.__enter__()
            otp_cm = tc.tile_pool(name="otp", bufs=3)
            otp = otp_cm.__enter__()
            pss_cm = tc.tile_pool(name="pss", bufs=2, space="PSUM")
            pss = pss_cm.__enter__()
            psy_cm = tc.tile_pool(name="psy", bufs=2, space="PSUM")
            psy = psy_cm.__enter__()
            psv_cm = tc.tile_pool(name="psv", bufs=2, space="PSUM")
            psv = psv_cm.__enter__()

            ybufs = {}
            pso = None

            def v_chain(t16):
                acc = psv.tile([128, FPC], F32, tag="pv")
                for kc in range(KC):
                    nc.tensor.matmul(
                        acc, XT[:, kc, 128 * t16:128 * t16 + 128],
                        WV[:, kc, :], start=(kc == 0), stop=(kc == KC - 1))
                nc.vector.tensor_copy(
                    out=VA[:, t16, :, 0:64],
                    in_=acc.rearrange("p (h e) -> p h e", e=64))

            def s_pieces(h, qh, v_list=()):
                """Generator: one S/exp piece per next() (+ V chains)."""
                p, r0 = h // 2, 64 * (h % 2)
                q0 = 1024 * qh
                v_list = list(v_list)
                pts = {}
                for kc in range(8 * (qh + 1)):
                    qs = max(q0, 128 * kc)
                    cols = q0 + 1024 - qs
                    sp = pss.tile([128, 1024], F32, tag="sp")
                    off = 0
                    while off < cols:
                        cw = min(512 - (off % 512), cols - off)
                        nc.tensor.matmul(
                            sp[:, off:off + cw],
                            KT[p][r0:r0 + 64, 128 * kc:128 * kc + 128],
                            QT[p][r0:r0 + 64, qs + off:qs + off + cw],
                            start=True, stop=True)
                        off += cw
                    pt = ptp.tile([128, 1024], BF16, tag="pt")
                    nc.scalar.activation(out=pt[:, 0:cols], in_=sp[:, 0:cols],
                                         func=EXP, scale=CON[:, 0:1])
                    if 128 * kc >= q0:  # diagonal block leads the piece
                        nc.vector.tensor_mul(pt[:, 0:128], pt[:, 0:128], MSK)
                    pts[kc] = (pt, qs)
                    if v_list:
                        v_chain(v_list.pop(0))
                    yield pts

            def pv_chains(h, qh, pts):
                """Generator: one PV accumulation chain (q-block) per next().
                Chains are sequential per PSUM bank: a start=True in a bank
                wipes still-open chains there, so chain qb fully closes
                before chain qb+1 opens."""
                q0 = 1024 * qh
                yqs = [psy.tile([128, 4, 128], F32, tag="yq",
                                name=f"yq_{h}_{qh}_{w}") for w in range(2)]
                for qb in range(8):
                    for kc in range(8 * qh + qb + 1):
                        pt, qs = pts[kc]
                        off = 128 * qb + q0 - qs
                        nc.tensor.matmul(
                            yqs[qb // 4][:, qb % 4, 0:65],
                            pt[:, off:off + 128], VA[:, kc, h, :],
                            start=(kc == 0), stop=(kc == 8 * qh + qb))
                    yield
                # drain: normalize each 128-q block by its exp-sum
                p = h // 2
                for w in range(2):
                    yq = yqs[w]
                    rec = recp.tile([128, 4], F32, tag="rec")
                    nc.vector.reciprocal(
                        out=rec,
                        in_=yq[:, :, 64:65].rearrange("p a e -> p (a e)"))
                    for qb4 in range(4):
                        qbg = 8 * qh + 4 * w + qb4
                        if h % 2 == 0:
                            ybufs[(p, qbg)] = ybp.tile(
                                [128, 2, 64], BF16, tag="yb",
                                name=f"yb_{p}_{qbg}")
                        nc.vector.tensor_scalar_mul(
                            ybufs[(p, qbg)][:, h % 2, :],
                            yq[:, qb4, 0:64], rec[:, qb4:qb4 + 1])
                if h % 2 == 1:
                    for qb in range(8):
                        qbg = 8 * qh + qb
                        yb = ybufs.pop((p, qbg))
                        nc.sync.dma_start_transpose(
                            out=AT[p][:, 128 * qbg:128 * qbg + 128],
                            in_=yb[:, :, :].rearrange("p a e -> p (a e)"))
                yield

            def oproj(t16, on_act=False):
                ot = otp.tile([128, D], BF16, tag="ot")
                yo = pso.tile([128, 2, 512], F32, tag="yo")
                for half in range(2):
                    for ft in range(2):
                        nc.tensor.matmul(
                            yo[:, half, :],
                            AT[ft][:, 128 * t16:128 * t16 + 128],
                            WOC[:, ft, 512 * half:512 * half + 512],
                            start=(ft == 0), stop=(ft == 1))
                if on_act:
                    nc.scalar.activation(
                        out=ot, in_=yo.rearrange("p a b -> p (a b)"),
                        func=CPY, scale=CON[:, 1:2])
                else:
                    nc.vector.tensor_scalar_mul(
                        ot, yo.rearrange("p a b -> p (a b)"), CON[:, 1:2])
                nc.sync.dma_start(out=yp_d[128 * t16:128 * t16 + 128, :],
                                  in_=ot)

            def drive(chain_gen, piece_gen, opro=(), ratio=1):
                """Round-robin: PV chains of head h with S pieces of head
                h+1 (and O-proj chunks) until all exhausted."""
                opro = list(opro)
                last = None
                c_done = chain_gen is None
                p_done = piece_gen is None
                while not (c_done and p_done and not opro):
                    if not c_done:
                        try:
                            next(chain_gen)
                        except StopIteration:
                            c_done = True
                    if opro:
                        oproj(opro.pop(0))
                    if not p_done:
                        for _ in range(ratio):
                            try:
                                last = next(piece_gen)
                            except StopIteration:
                                p_done = True
                                break
                return last

            # ---- pipelined schedule ----
            HEADS = [(h, qh) for qh in (0, 1) for h in range(4)]
            chain_gen = None
            pso_cm = None
            for h, qh in HEADS:
                if (h, qh) == (2, 1):
                    # all V chains emitted by end of (1,1): psv -> pso
                    psv_cm.__exit__(None, None, None)
                    pso_cm = tc.tile_pool(name="pso", bufs=1, space="PSUM")
                    pso = pso_cm.__enter__()
                v_list = ()
                if h == 0:
                    v_list = range(8 * qh, 8 * qh + 4)
                elif h == 1:
                    v_list = range(8 * qh + 4, 8 * qh + 8)
                piece_gen = s_pieces(h, qh, v_list=v_list)
                opro = ()
                if (h, qh) == (2, 1):
                    opro = range(0, 4)
                elif (h, qh) == (3, 1):
                    opro = range(4, 8)
                pts = drive(chain_gen, piece_gen, opro, ratio=qh + 1)
                chain_gen = pv_chains(h, qh, pts)
            drive(chain_gen, None)
            # tail: widen the O-proj PSUM pool so the last 8 chunks pipeline
            pso_cm.__exit__(None, None, None)
            psy_cm.__exit__(None, None, None)
            pss_cm.__exit__(None, None, None)
            pso_cm = tc.tile_pool(name="pso2", bufs=4, space="PSUM")
            pso = pso_cm.__enter__()
            for t16 in range(8, 16):
                oproj(t16, on_act=True)

            pso_cm.__exit__(None, None, None)
            otp_cm.__exit__(None, None, None)
            recp_cm.__exit__(None, None, None)
            ybp_cm.__exit__(None, None, None)
            ptp_cm.__exit__(None, None, None)

    _split_excess_waits(nc)
    return nc


_NC = None
_LAST_INMAPS = None


def _get_nc():
    global _NC
    if _NC is None:
        _NC = _build()
    return _NC


def _ternary_signs(w):
    """Mirror reference ternary_weight: returns (signs in {-1,0,1}, scale)."""
    try:
        import jax
        import jax.numpy as jnp
        cpu = jax.devices("cpu")[0]
        with jax.default_device(cpu):
            wj = jnp.asarray(np.asarray(w, dtype=np.float32))
            scale = jnp.mean(jnp.abs(wj))
            signs = jnp.round(jnp.clip(wj / (scale + 1e-8), -1.0, 1.0))
            return np.asarray(signs, dtype=np.float32), float(scale)
    except Exception:
        w = np.asarray(w, dtype=np.float32)
        scale = np.float32(np.mean(np.abs(w)))
        signs = np.round(np.clip(w / (scale + np.float32(1e-8)), -1.0, 1.0))
        return signs.astype(np.float32), float(scale)


def _rope_tables():
    inv = (1.0 / (10000.0 ** (np.arange(0, HD, 2, dtype=np.float32) / HD))
           ).astype(np.float32)                      # [32]
    t = np.arange(T, dtype=np.float32)
    fr = np.outer(t, inv).astype(np.float32)         # [T, 32]
    cos1 = np.cos(fr).astype(np.float32)
    sin1 = np.sin(fr).astype(np.float32)
    # rows: d in 0..63 (freq d%32), tiled for 2 heads -> 128 rows
    cosd = np.concatenate([cos1, cos1], axis=1).T    # [64, T]
    sind = np.concatenate([sin1, sin1], axis=1).T
    sgn = np.ones((HD, 1), dtype=np.float32)
    sgn[:HD // 2] = -1.0
    cos2 = np.tile(cosd, (2, 1)).astype(NPBF)                # [128, T]
    sins = np.tile(sind * sgn, (2, 1)).astype(NPBF)
    return cos2, sins


def kernel(x, Wq, Wk, Wv, Wo, mask):
    global _LAST_INMAPS
    x = np.asarray(x, dtype=np.float32)
    mask = np.asarray(mask)
    assert np.array_equal(
        np.asarray(mask[0, 0], dtype=np.int32),
        np.tril(np.ones((T, T), dtype=np.int32))), "non-causal mask"

    qs, sq = _ternary_signs(Wq)
    ks, sk = _ternary_signs(Wk)
    vs, sv = _ternary_signs(Wv)
    os_, so = _ternary_signs(Wo)
    cos2, sins = _rope_tables()
    mvals = np.triu(np.ones((128, 128), dtype=np.float32)).astype(NPBF)
    consts = np.zeros((128, 2), dtype=np.float32)
    consts[:, 0] = np.float32(sq) * np.float32(sk) * np.float32(0.125)
    consts[:, 1] = np.float32(sv) * np.float32(so)

    in_maps = []
    for c in range(NCORES):
        b, g = c // 4, c % 4
        fsl = slice(FPC * g, FPC * g + FPC)
        in_maps.append({
            "xt": np.ascontiguousarray(x[b].T).astype(NPBF),
            "wqt": np.ascontiguousarray(qs[fsl].T).astype(NPBF),
            "wkt": np.ascontiguousarray(ks[fsl].T).astype(NPBF),
            "wvt": np.ascontiguousarray(vs[fsl].T).astype(NPBF),
            "woc": np.ascontiguousarray(os_[:, fsl].T).astype(NPBF),
            "cos2": cos2,
            "sins": sins,
            "maskm": mvals,
            "consts": consts,
        })
    _LAST_INMAPS = in_maps

    res = run_bass_kernel_spmd(_get_nc(), in_maps,
                               core_ids=list(range(NCORES)))
    out = np.zeros((B, T, D), dtype=np.float32)
    for b in range(B):
        acc = np.zeros((T, D), dtype=np.float32)
        for g in range(4):
            acc += np.asarray(res.results[4 * b + g]["yp"],
                              dtype=np.float32)
        out[b] = acc
    return out


def bench(trace=True):
    """Re-run last inputs with NTFF tracing; returns BassKernelResults."""
    assert _LAST_INMAPS is not None, "call kernel() first"
    return run_bass_kernel_spmd(_get_nc(), _LAST_INMAPS,
                                core_ids=list(range(NCORES)), trace=trace)


# revision 43
# speedup vs baseline: 1.4216x; 1.0961x over previous
"""BinarySelfAttention Trainium2 kernel (8-core SPMD), v2.

Strategy: shard (batch, head-group): core c -> batch c//4, heads 4*(c%4)..+3.
Each core computes ternary-projected QKV for its 4 heads (bf16 data path,
ternary signs exact in bf16), RoPE, causal attention in S^T orientation
(keys on partitions), a FLIPPED PV matmul (exp(S^T) chunks stationary, V
moving at 65 cols -> half the PE time of the 65-row orientation), per-
partition softmax normalization (no DRAM bounce), DMA-engine transposes of
the normalized y into [feature, T] layout, and a partial output projection
against its Wo column slice. Host sums the 4 bf16 partials per batch in f32.

Ternary scales are folded into the exp() scale (sq*sk/8) and the output
eviction (sv*so), passed as runtime data so the program is input-independent.
"""
import numpy as np
import ml_dtypes

import concourse.bass as bass
import concourse.mybir as mybir
import concourse.tile as tile
from concourse.bass_utils import run_bass_kernel_spmd

F32 = mybir.dt.float32
BF16 = mybir.dt.bfloat16
NPBF = ml_dtypes.bfloat16

B, T, D, H = 2, 2048, 1024, 16
HD = 64            # head dim
HPC = 4            # heads per core
FPC = HPC * HD     # features per core (256)
NCORES = 8
KC = D // 128      # 8 contraction chunks for projections


def _split_excess_waits(nc):
    """walrus wait-slot limits: 1 for most instructions, 0 for the DMA
    transpose, 1 for TensorScalarPtr; hoist excess onto same-queue NoOps."""
    LIMS = {"InstDmaTransposeAnt": 0, "InstTensorScalarPtr": 1}
    n = 0
    for f in nc.m.functions:
        for bb in f.blocks:
            new_insts = []
            for inst in bb.instructions:
                si = getattr(inst, 'sync_info', None)
                lim = LIMS.get(type(inst).__name__, 1)
                if si is not None and si.on_wait and len(si.on_wait) > lim:
                    waits = list(si.on_wait)
                    extra, keep = (waits, []) if lim == 0 else \
                        (waits[:-lim], waits[-lim:])
                    for j, w in enumerate(extra):
                        new_insts.append(mybir.InstNoOp(
                            name=f"{inst.name}-wsplit{j}",
                            engine=inst.engine,
                            sync_info=mybir.SyncInfo(on_wait=[w], on_update=[]),
                            bass_nofuse=True,
                        ))
                        n += 1
                    inst.sync_info = mybir.SyncInfo(
                        on_wait=keep, on_update=si.on_update)
                new_insts.append(inst)
            bb.instructions[:] = new_insts
    return n


def _build():
    nc = bass.Bass("TRN2", target_bir_lowering=False, debug=False,
                   num_devices=NCORES)
    xt_d = nc.dram_tensor("xt", [D, T], BF16, kind="ExternalInput")
    wq_d = nc.dram_tensor("wqt", [D, FPC], BF16, kind="ExternalInput")
    wk_d = nc.dram_tensor("wkt", [D, FPC], BF16, kind="ExternalInput")
    wv_d = nc.dram_tensor("wvt", [D, FPC], BF16, kind="ExternalInput")
    wo_d = nc.dram_tensor("woc", [FPC, D], BF16, kind="ExternalInput")
    cos_d = nc.dram_tensor("cos2", [128, T], BF16, kind="ExternalInput")
    sin_d = nc.dram_tensor("sins", [128, T], BF16, kind="ExternalInput")
    msk_d = nc.dram_tensor("maskm", [128, 128], BF16, kind="ExternalInput")
    con_d = nc.dram_tensor("consts", [128, 2], F32, kind="ExternalInput")
    yp_d = nc.dram_tensor("yp", [T, D], BF16, kind="ExternalOutput")

    EXP = mybir.ActivationFunctionType.Exp
    CPY = mybir.ActivationFunctionType.Copy

    with tile.TileContext(nc) as tc:
        with tc.tile_pool(name="main", bufs=1) as mp:
            CON = mp.tile([128, 2], F32)
            MSK = mp.tile([128, 128], BF16)
            XT = mp.tile([128, KC, T], BF16)
            COS = mp.tile([128, T], BF16)
            SIN = mp.tile([128, T], BF16)
            QT = [mp.tile([128, T], BF16, name=f"qt{i}") for i in range(2)]
            KT = [mp.tile([128, T], BF16, name=f"kt{i}") for i in range(2)]
            VA = mp.tile([128, 16, HPC, 65], BF16)
            AT = [mp.tile([128, T], BF16, name=f"at{i}") for i in range(2)]
            WOC = mp.tile([128, 2, D], BF16)
            ONES = mp.tile([128, 64], BF16)

            # DMA issue order drives the serial DMA device: x chunks first
            # (kc-streaming projections), each ternary weight as ONE batched
            # SWDGE transfer, Wo/consts deferred (needed late).
            WQ = mp.tile([128, KC, FPC], BF16, name="wq")
            WK = mp.tile([128, KC, FPC], BF16, name="wk")
            WV = mp.tile([128, KC, FPC], BF16, name="wv")
            nc.gpsimd.dma_start(
                out=WQ[:, 0:4, :],
                in_=wq_d[0:512, :].rearrange("(kc p) f -> p kc f", p=128))
            nc.gpsimd.dma_start(
                out=WK[:, 0:4, :],
                in_=wk_d[0:512, :].rearrange("(kc p) f -> p kc f", p=128))
            nc.sync.dma_start(out=XT[:, 0, 0:1024],
                              in_=xt_d[0:128, 0:1024])
            nc.sync.dma_start(out=XT[:, 0, 1024:2048],
                              in_=xt_d[0:128, 1024:2048])
            nc.gpsimd.dma_start(
                out=WQ[:, 4:8, :],
                in_=wq_d[512:1024, :].rearrange("(kc p) f -> p kc f", p=128))
            nc.gpsimd.dma_start(
                out=WK[:, 4:8, :],
                in_=wk_d[512:1024, :].rearrange("(kc p) f -> p kc f", p=128))
            for kc in range(1, KC):
                eng = nc.sync if kc % 2 == 0 else nc.scalar
                eng.dma_start(out=XT[:, kc, :],
                              in_=xt_d[128 * kc:128 * kc + 128, :])
            nc.gpsimd.dma_start(
                out=WV[:, :, :],
                in_=wv_d.rearrange("(kc p) f -> p kc f", p=128))
            nc.scalar.dma_start(out=COS, in_=cos_d[:, :])
            nc.scalar.dma_start(out=SIN, in_=sin_d[:, :])
            nc.sync.dma_start(out=CON, in_=con_d[:, :])
            nc.sync.dma_start(out=MSK, in_=msk_d[:, :])
            nc.vector.memset(ONES, 1.0)
            ones_view = VA[:, :, :, 64:65].rearrange("p a h e -> p (a h e)")
            nc.vector.tensor_copy(out=ones_view, in_=ONES[:, 0:64])
            for ft in range(2):
                nc.scalar.dma_start(out=WOC[:, ft, :],
                                    in_=wo_d[128 * ft:128 * ft + 128, :])

            def proj_pair(dt_i, psqk):
                # kc-streaming Q+K for ONE head pair (8 matmuls/kc matches
                # the x-chunk DMA pace); evictions split Act/DVE
                accs = [psqk.tile([128, 512], F32, tag=f"pq{w}{i}",
                                  name=f"acc{dt_i}{w}{i}")
                        for w in range(2) for i in range(4)]
                for kc in range(KC):
                    for w, wt in enumerate((WQ, WK)):
                        for tch in range(4):
                            nc.tensor.matmul(
                                accs[4 * w + tch],
                                wt[:, kc, 128 * dt_i:128 * dt_i + 128],
                                XT[:, kc, 512 * tch:512 * tch + 512],
                                start=(kc == 0), stop=(kc == KC - 1))
                for w, dest in enumerate((QT, KT)):
                    for tch in range(4):
                        eng = (nc.scalar.copy if tch % 2 == 0
                               else nc.vector.tensor_copy)
                        eng(out=dest[dt_i][:, 512 * tch:512 * tch + 512],
                            in_=accs[4 * w + tch])

            ROT = [mp.tile([128, T], BF16, name=f"rot{i}") for i in range(2)]

            def rope(dest, dt_i, rot_i):
                dst = dest[dt_i]
                rot = ROT[rot_i]
                for g in range(2):
                    b0 = 64 * g
                    nc.gpsimd.dma_start(out=rot[b0:b0 + 32, :],
                                        in_=dst[b0 + 32:b0 + 64, :])
                    nc.gpsimd.dma_start(out=rot[b0 + 32:b0 + 64, :],
                                        in_=dst[b0:b0 + 32, :])
                nc.vector.tensor_mul(rot, rot, SIN)
                nc.vector.tensor_mul(dst, dst, COS)
                nc.vector.tensor_add(dst, dst, rot)

            with tc.tile_pool(name="psqk", bufs=1, space="PSUM") as psqk:
                # pair 0 fully first so attention on heads 0,1 starts early
                proj_pair(0, psqk)
                rope(QT, 0, 0)
                rope(KT, 0, 1)
                proj_pair(1, psqk)

            # ------------- Phase 2: attention (+V proj, +O proj) ---------
            # Software-pipelined heads: head h's PV chains interleave with
            # head h+1's S/exp pieces so the Act engine (exp) stays fed by
            # the in-order PE stream; V chains and O-proj chunks fill
            # leftover PE slack.
            ptp_cm = tc.tile_pool(name="ptp", bufs=36)
            ptp = ptp_cm.__enter__()
            ybp_cm = tc.tile_pool(name="ybp", bufs=20)
            ybp = ybp_cm.__enter__()
            recp_cm = tc.tile_pool(name="recp", bufs=4)
            recp = recp_cm.__enter__()
            otp_cm = tc.tile_pool(name="otp", bufs=4)
            otp = otp_cm.__enter__()
            pss_cm = tc.tile_pool(name="pss", bufs=2, space="PSUM")
            pss = pss_cm.__enter__()
            psy_cm = tc.tile_pool(name="psy", bufs=2, space="PSUM")
            psy = psy_cm.__enter__()
            psv_cm = tc.tile_pool(name="psv", bufs=2, space="PSUM")
            psv = psv_cm.__enter__()

            ybufs = {}
            pso = None

            def v_chain(t16):
                acc = psv.tile([128, FPC], F32, tag="pv")
                for kc in range(KC):
                    nc.tensor.matmul(
                        acc, XT[:, kc, 128 * t16:128 * t16 + 128],
                        WV[:, kc, :], start=(kc == 0), stop=(kc == KC - 1))
                nc.vector.tensor_copy(
                    out=VA[:, t16, :, 0:64],
                    in_=acc.rearrange("p (h e) -> p h e", e=64))

            def s_pieces(h, qh, v_list=()):
                """Generator: one S/exp piece per next() (+ V chains)."""
                p, r0 = h // 2, 64 * (h % 2)
                q0 = 1024 * qh
                v_list = list(v_list)
                pts = {}
                for kc in range(8 * (qh + 1)):
                    qs = max(q0, 128 * kc)
                    cols = q0 + 1024 - qs
                    sp = pss.tile([128, 1024], F32, tag="sp")
                    off = 0
                    while off < cols:
                        cw = min(512 - (off % 512), cols - off)
                        nc.tensor.matmul(
                            sp[:, off:off + cw],
                            KT[p][r0:r0 + 64, 128 * kc:128 * kc + 128],
                            QT[p][r0:r0 + 64, qs + off:qs + off + cw],
                            start=True, stop=True)
                        off += cw
                    pt = ptp.tile([128, 1024], BF16, tag="pt")
                    nc.scalar.activation(out=pt[:, 0:cols], in_=sp[:, 0:cols],
                                         func=EXP, scale=CON[:, 0:1])
                    if 128 * kc >= q0:  # diagonal block leads the piece
                        nc.gpsimd.tensor_mul(pt[:, 0:128], pt[:, 0:128], MSK)
                    pts[kc] = (pt, qs)
                    if v_list:
                        v_chain(v_list.pop(0))
                    yield pts

            def pv_chains(h, qh, pts, post_qb=None):
                """Generator: one PV accumulation chain (q-block) per next().
                Chains are sequential per PSUM bank: a start=True in a bank
                wipes still-open chains there, so chain qb fully closes
                before chain qb+1 opens. Each 4-block window drains
                (normalize / transpose / post hook) as soon as it completes."""
                q0 = 1024 * qh
                p = h // 2
                yqs = [psy.tile([128, 4, 128], F32, tag="yq",
                                name=f"yq_{h}_{qh}_{w}") for w in range(2)]
                for qb in range(8):
                    for kc in range(8 * qh + qb + 1):
                        pt, qs = pts[kc]
                        off = 128 * qb + q0 - qs
                        nc.tensor.matmul(
                            yqs[qb // 4][:, qb % 4, 0:65],
                            pt[:, off:off + 128], VA[:, kc, h, :],
                            start=(kc == 0), stop=(kc == 8 * qh + qb))
                    if qb % 4 == 3:
                        w = qb // 4
                        yq = yqs[w]
                        rec = recp.tile([128, 4], F32, tag="rec")
                        nc.vector.reciprocal(
                            out=rec,
                            in_=yq[:, :, 64:65].rearrange("p a e -> p (a e)"))
                        for qb4 in range(4):
                            qbg = 8 * qh + 4 * w + qb4
                            if h % 2 == 0:
                                ybufs[(p, qbg)] = ybp.tile(
                                    [128, 2, 64], BF16, tag="yb",
                                    name=f"yb_{p}_{qbg}")
                            nc.vector.tensor_scalar_mul(
                                ybufs[(p, qbg)][:, h % 2, :],
                                yq[:, qb4, 0:64], rec[:, qb4:qb4 + 1])
                        if h % 2 == 1:
                            for qb4 in range(4):
                                qbg = 8 * qh + 4 * w + qb4
                                yb = ybufs.pop((p, qbg))
                                nc.sync.dma_start_transpose(
                                    out=AT[p][:, 128 * qbg:128 * qbg + 128],
                                    in_=yb[:, :, :].rearrange(
                                        "p a e -> p (a e)"))
                            if post_qb is not None:
                                for qb4 in range(4):
                                    post_qb(8 * qh + 4 * w + qb4)
                    yield

            def oproj(t16, split_evict=False):
                ot = otp.tile([128, D], BF16, tag="ot")
                for half in range(2):
                    yo = pso.tile([128, 512], F32, tag="yo")
                    for ft in range(2):
                        nc.tensor.matmul(
                            yo, AT[ft][:, 128 * t16:128 * t16 + 128],
                            WOC[:, ft, 512 * half:512 * half + 512],
                            start=(ft == 0), stop=(ft == 1))
                    if split_evict and half == 1:
                        nc.scalar.activation(
                            out=ot[:, 512 * half:512 * half + 512], in_=yo,
                            func=CPY, scale=CON[:, 1:2])
                    else:
                        nc.vector.tensor_scalar_mul(
                            ot[:, 512 * half:512 * half + 512], yo,
                            CON[:, 1:2])
                nc.sync.dma_start(out=yp_d[128 * t16:128 * t16 + 128, :],
                                  in_=ot)

            def drive(chain_gen, piece_gen, opro=(), ratio=1):
                """Round-robin: PV chains of head h with S pieces of head
                h+1 (and O-proj chunks) until all exhausted."""
                opro = list(opro)
                last = None
                c_done = chain_gen is None
                p_done = piece_gen is None
                while not (c_done and p_done and not opro):
                    if not c_done:
                        try:
                            next(chain_gen)
                        except StopIteration:
                            c_done = True
                    if opro:
                        oproj(opro.pop(0))
                    if not p_done:
                        for _ in range(ratio):
                            try:
                                last = next(piece_gen)
                            except StopIteration:
                                p_done = True
                                break
                return last

            # ---- pipelined schedule ----
            HEADS = [(h, qh) for qh in (0, 1) for h in range(4)]
            chain_gen = None
            pso_cm = None
            for h, qh in HEADS:
                if (h, qh) == (2, 0):
                    # pair-1 rope deferred here so the early DVE queue
                    # (V evicts feed the PV chains) is not blocked by it
                    rope(QT, 1, 0)
                    rope(KT, 1, 1)
                if (h, qh) == (2, 1):
                    # all V chains emitted by end of (1,1): psv -> pso
                    psv_cm.__exit__(None, None, None)
                    pso_cm = tc.tile_pool(name="pso", bufs=2, space="PSUM")
                    pso = pso_cm.__enter__()
                v_list = ()
                if h == 0:
                    v_list = range(8 * qh, 8 * qh + 4)
                elif h == 1:
                    v_list = range(8 * qh + 4, 8 * qh + 8)
                piece_gen = s_pieces(h, qh, v_list=v_list)
                opro = ()
                if (h, qh) == (2, 1):
                    opro = range(0, 4)
                elif (h, qh) == (3, 1):
                    opro = range(4, 8)
                pts = drive(chain_gen, piece_gen, opro, ratio=qh + 1)
                chain_gen = pv_chains(h, qh, pts)
            drive(chain_gen, None)
            # tail: widen the O-proj PSUM pool so the last 8 chunks pipeline
            pso_cm.__exit__(None, None, None)
            psy_cm.__exit__(None, None, None)
            pss_cm.__exit__(None, None, None)
            pso_cm = tc.tile_pool(name="pso2", bufs=4, space="PSUM")
            pso = pso_cm.__enter__()
            for t16 in range(8, 16):
                oproj(t16, split_evict=True)
            pso_cm.__exit__(None, None, None)
            otp_cm.__exit__(None, None, None)
            recp_cm.__exit__(None, None, None)
            ybp_cm.__exit__(None, None, None)
            ptp_cm.__exit__(None, None, None)

    _split_excess_waits(nc)
    return nc


_NC = None
_LAST_INMAPS = None


def _get_nc():
    global _NC
    if _NC is None:
        _NC = _build()
    return _NC


def _ternary_signs(w):
    """Mirror reference ternary_weight: returns (signs in {-1,0,1}, scale)."""
    try:
        import jax
        import jax.numpy as jnp
        cpu = jax.devices("cpu")[0]
        with jax.default_device(cpu):
            wj = jnp.asarray(np.asarray(w, dtype=np.float32))
            scale = jnp.mean(jnp.abs(wj))
            signs = jnp.round(jnp.clip(wj / (scale + 1e-8), -1.0, 1.0))
            return np.asarray(signs, dtype=np.float32), float(scale)
    except Exception:
        w = np.asarray(w, dtype=np.float32)
        scale = np.float32(np.mean(np.abs(w)))
        signs = np.round(np.clip(w / (scale + np.float32(1e-8)), -1.0, 1.0))
        return signs.astype(np.float32), float(scale)


def _rope_tables():
    inv = (1.0 / (10000.0 ** (np.arange(0, HD, 2, dtype=np.float32) / HD))
           ).astype(np.float32)                      # [32]
    t = np.arange(T, dtype=np.float32)
    fr = np.outer(t, inv).astype(np.float32)         # [T, 32]
    cos1 = np.cos(fr).astype(np.float32)
    sin1 = np.sin(fr).astype(np.float32)
    # rows: d in 0..63 (freq d%32), tiled for 2 heads -> 128 rows
    cosd = np.concatenate([cos1, cos1], axis=1).T    # [64, T]
    sind = np.concatenate([sin1, sin1], axis=1).T
    sgn = np.ones((HD, 1), dtype=np.float32)
    sgn[:HD // 2] = -1.0
    cos2 = np.tile(cosd, (2, 1)).astype(NPBF)                # [128, T]
    sins = np.tile(sind * sgn, (2, 1)).astype(NPBF)
    return cos2, sins


def kernel(x, Wq, Wk, Wv, Wo, mask):
    global _LAST_INMAPS
    x = np.asarray(x, dtype=np.float32)
    mask = np.asarray(mask)
    assert np.array_equal(
        np.asarray(mask[0, 0], dtype=np.int32),
        np.tril(np.ones((T, T), dtype=np.int32))), "non-causal mask"

    qs, sq = _ternary_signs(Wq)
    ks, sk = _ternary_signs(Wk)
    vs, sv = _ternary_signs(Wv)
    os_, so = _ternary_signs(Wo)
    cos2, sins = _rope_tables()
    mvals = np.triu(np.ones((128, 128), dtype=np.float32)).astype(NPBF)
    consts = np.zeros((128, 2), dtype=np.float32)
    consts[:, 0] = np.float32(sq) * np.float32(sk) * np.float32(0.125)
    consts[:, 1] = np.float32(sv) * np.float32(so)

    in_maps = []
    for c in range(NCORES):
        b, g = c // 4, c % 4
        fsl = slice(FPC * g, FPC * g + FPC)
        in_maps.append({
            "xt": np.ascontiguousarray(x[b].T).astype(NPBF),
            "wqt": np.ascontiguousarray(qs[fsl].T).astype(NPBF),
            "wkt": np.ascontiguousarray(ks[fsl].T).astype(NPBF),
            "wvt": np.ascontiguousarray(vs[fsl].T).astype(NPBF),
            "woc": np.ascontiguousarray(os_[:, fsl].T).astype(NPBF),
            "cos2": cos2,
            "sins": sins,
            "maskm": mvals,
            "consts": consts,
        })
    _LAST_INMAPS = in_maps

    res = run_bass_kernel_spmd(_get_nc(), in_maps,
                               core_ids=list(range(NCORES)))
    out = np.zeros((B, T, D), dtype=np.float32)
    for b in range(B):
        acc = np.zeros((T, D), dtype=np.float32)
        for g in range(4):
            acc += np.asarray(res.results[4 * b + g]["yp"],
                              dtype=np.float32)
        out[b] = acc
    return out


def bench(trace=True):
    """Re-run last inputs with NTFF tracing; returns BassKernelResults."""
    assert _LAST_INMAPS is not None, "call kernel() first"
    return run_bass_kernel_spmd(_get_nc(), _LAST_INMAPS,
                                core_ids=list(range(NCORES)), trace=trace)


# revision 59
# speedup vs baseline: 1.4991x; 1.0545x over previous
"""BinarySelfAttention Trainium2 kernel (8-core SPMD), v2.

Strategy: shard (batch, head-group): core c -> batch c//4, heads 4*(c%4)..+3.
Each core computes ternary-projected QKV for its 4 heads (bf16 data path,
ternary signs exact in bf16), RoPE, causal attention in S^T orientation
(keys on partitions), a FLIPPED PV matmul (exp(S^T) chunks stationary, V
moving at 65 cols -> half the PE time of the 65-row orientation), per-
partition softmax normalization (no DRAM bounce), DMA-engine transposes of
the normalized y into [feature, T] layout, and a partial output projection
against its Wo column slice. Host sums the 4 bf16 partials per batch in f32.

Ternary scales are folded into the exp() scale (sq*sk/8) and the output
eviction (sv*so), passed as runtime data so the program is input-independent.
"""
import numpy as np
import ml_dtypes

import concourse.bass as bass
import concourse.mybir as mybir
import concourse.tile as tile
from concourse.bass_utils import run_bass_kernel_spmd

F32 = mybir.dt.float32
BF16 = mybir.dt.bfloat16
NPBF = ml_dtypes.bfloat16

B, T, D, H = 2, 2048, 1024, 16
HD = 64            # head dim
HPC = 4            # heads per core
FPC = HPC * HD     # features per core (256)
NCORES = 8
KC = D // 128      # 8 contraction chunks for projections


def _split_excess_waits(nc):
    """walrus wait-slot limits: 1 for most instructions, 0 for the DMA
    transpose, 1 for TensorScalarPtr; hoist excess onto same-queue NoOps."""
    LIMS = {"InstDmaTransposeAnt": 0, "InstTensorScalarPtr": 1}
    n = 0
    for f in nc.m.functions:
        for bb in f.blocks:
            new_insts = []
            for inst in bb.instructions:
                si = getattr(inst, 'sync_info', None)
                lim = LIMS.get(type(inst).__name__, 1)
                if si is not None and si.on_wait and len(si.on_wait) > lim:
                    waits = list(si.on_wait)
                    extra, keep = (waits, []) if lim == 0 else \
                        (waits[:-lim], waits[-lim:])
                    for j, w in enumerate(extra):
                        new_insts.append(mybir.InstNoOp(
                            name=f"{inst.name}-wsplit{j}",
                            engine=inst.engine,
                            sync_info=mybir.SyncInfo(on_wait=[w], on_update=[]),
                            bass_nofuse=True,
                        ))
                        n += 1
                    inst.sync_info = mybir.SyncInfo(
                        on_wait=keep, on_update=si.on_update)
                new_insts.append(inst)
            bb.instructions[:] = new_insts
    return n


def _build():
    nc = bass.Bass("TRN2", target_bir_lowering=False, debug=False,
                   num_devices=NCORES)
    xt_d = nc.dram_tensor("xt", [D, T], BF16, kind="ExternalInput")
    wq_d = nc.dram_tensor("wqt", [D, FPC], BF16, kind="ExternalInput")
    wk_d = nc.dram_tensor("wkt", [D, FPC], BF16, kind="ExternalInput")
    wv_d = nc.dram_tensor("wvt", [D, FPC], BF16, kind="ExternalInput")
    wo_d = nc.dram_tensor("woc", [FPC, D], BF16, kind="ExternalInput")
    cos_d = nc.dram_tensor("cos2", [128, T], BF16, kind="ExternalInput")
    sin_d = nc.dram_tensor("sins", [128, T], BF16, kind="ExternalInput")
    msk_d = nc.dram_tensor("maskm", [128, 128], BF16, kind="ExternalInput")
    con_d = nc.dram_tensor("consts", [128, 2], F32, kind="ExternalInput")
    yp_d = nc.dram_tensor("yp", [T, D], BF16, kind="ExternalOutput")

    EXP = mybir.ActivationFunctionType.Exp
    CPY = mybir.ActivationFunctionType.Copy

    with tile.TileContext(nc) as tc:
        with tc.tile_pool(name="main", bufs=1) as mp:
            CON = mp.tile([128, 2], F32)
            MSK = mp.tile([128, 128], BF16)
            XT = mp.tile([128, KC, T], BF16)
            COS = mp.tile([128, T], BF16)
            SIN = mp.tile([128, T], BF16)
            QT = [mp.tile([128, T], BF16, name=f"qt{i}") for i in range(2)]
            KT = [mp.tile([128, T], BF16, name=f"kt{i}") for i in range(2)]
            VA = mp.tile([128, 16, HPC, 65], BF16)
            AT = [mp.tile([128, T], BF16, name=f"at{i}") for i in range(2)]
            WOC = mp.tile([128, 2, D], BF16)
            ONES = mp.tile([128, 64], BF16)

            # DMA issue order drives the serial DMA device: x chunks first
            # (kc-streaming projections), each ternary weight as ONE batched
            # SWDGE transfer, Wo/consts deferred (needed late).
            WQ = mp.tile([128, KC, FPC], BF16, name="wq")
            WK = mp.tile([128, KC, FPC], BF16, name="wk")
            WV = mp.tile([128, KC, FPC], BF16, name="wv")
            nc.sync.dma_start(
                out=WQ[:, 0:4, :],
                in_=wq_d[0:512, :].rearrange("(kc p) f -> p kc f", p=128))
            nc.scalar.dma_start(
                out=WK[:, 0:4, :],
                in_=wk_d[0:512, :].rearrange("(kc p) f -> p kc f", p=128))
            nc.sync.dma_start(out=XT[:, 0, 0:1024],
                              in_=xt_d[0:128, 0:1024])
            nc.sync.dma_start(out=XT[:, 0, 1024:2048],
                              in_=xt_d[0:128, 1024:2048])
            nc.gpsimd.dma_start(
                out=WQ[:, 4:8, :],
                in_=wq_d[512:1024, :].rearrange("(kc p) f -> p kc f", p=128))
            nc.gpsimd.dma_start(
                out=WK[:, 4:8, :],
                in_=wk_d[512:1024, :].rearrange("(kc p) f -> p kc f", p=128))
            for kc in range(1, KC):
                eng = nc.sync if kc % 2 == 0 else nc.scalar
                eng.dma_start(out=XT[:, kc, :],
                              in_=xt_d[128 * kc:128 * kc + 128, :])
            nc.gpsimd.dma_start(
                out=WV[:, :, :],
                in_=wv_d.rearrange("(kc p) f -> p kc f", p=128))
            nc.scalar.dma_start(out=COS, in_=cos_d[:, :])
            nc.scalar.dma_start(out=SIN, in_=sin_d[:, :])
            nc.sync.dma_start(out=CON, in_=con_d[:, :])
            nc.sync.dma_start(out=MSK, in_=msk_d[:, :])
            nc.vector.memset(ONES, 1.0)
            ones_view = VA[:, :, :, 64:65].rearrange("p a h e -> p (a h e)")
            nc.vector.tensor_copy(out=ones_view, in_=ONES[:, 0:64])
            for ft in range(2):
                nc.scalar.dma_start(out=WOC[:, ft, :],
                                    in_=wo_d[128 * ft:128 * ft + 128, :])

            def proj_pair(dt_i, psqk):
                # kc-streaming Q+K for ONE head pair (8 matmuls/kc matches
                # the x-chunk DMA pace); evictions split Act/DVE
                accs = [psqk.tile([128, 512], F32, tag=f"pq{w}{i}",
                                  name=f"acc{dt_i}{w}{i}")
                        for w in range(2) for i in range(4)]
                for kc in range(KC):
                    for w, wt in enumerate((WQ, WK)):
                        for tch in range(4):
                            nc.tensor.matmul(
                                accs[4 * w + tch],
                                wt[:, kc, 128 * dt_i:128 * dt_i + 128],
                                XT[:, kc, 512 * tch:512 * tch + 512],
                                start=(kc == 0), stop=(kc == KC - 1))
                for w, dest in enumerate((QT, KT)):
                    for tch in range(4):
                        eng = (nc.scalar.copy if tch % 2 == 0
                               else nc.vector.tensor_copy)
                        eng(out=dest[dt_i][:, 512 * tch:512 * tch + 512],
                            in_=accs[4 * w + tch])

            ROT = [mp.tile([128, T], BF16, name=f"rot{i}") for i in range(2)]

            def rope(dest, dt_i, rot_i):
                dst = dest[dt_i]
                rot = ROT[rot_i]
                for g in range(2):
                    b0 = 64 * g
                    nc.gpsimd.dma_start(out=rot[b0:b0 + 32, :],
                                        in_=dst[b0 + 32:b0 + 64, :])
                    nc.gpsimd.dma_start(out=rot[b0 + 32:b0 + 64, :],
                                        in_=dst[b0:b0 + 32, :])
                nc.vector.tensor_mul(rot, rot, SIN)
                nc.vector.tensor_mul(dst, dst, COS)
                nc.vector.tensor_add(dst, dst, rot)

            with tc.tile_pool(name="psqk", bufs=1, space="PSUM") as psqk:
                # pair 0 fully first so attention on heads 0,1 starts early
                proj_pair(0, psqk)
                rope(QT, 0, 0)
                rope(KT, 0, 1)
                proj_pair(1, psqk)

            # ------------- Phase 2: attention (+V proj, +O proj) ---------
            # Software-pipelined heads: head h's PV chains interleave with
            # head h+1's S/exp pieces so the Act engine (exp) stays fed by
            # the in-order PE stream; V chains and O-proj chunks fill
            # leftover PE slack.
            ptp_cm = tc.tile_pool(name="ptp", bufs=40)
            ptp = ptp_cm.__enter__()
            ybp_cm = tc.tile_pool(name="ybp", bufs=28)
            ybp = ybp_cm.__enter__()
            recp_cm = tc.tile_pool(name="recp", bufs=6)
            recp = recp_cm.__enter__()
            otp_cm = tc.tile_pool(name="otp", bufs=8)
            otp = otp_cm.__enter__()
            pss_cm = tc.tile_pool(name="pss", bufs=2, space="PSUM")
            pss = pss_cm.__enter__()
            psy_cm = tc.tile_pool(name="psy", bufs=2, space="PSUM")
            psy = psy_cm.__enter__()
            psv_cm = tc.tile_pool(name="psv", bufs=2, space="PSUM")
            psv = psv_cm.__enter__()

            ybufs = {}
            pso = None

            def v_chain(t16):
                acc = psv.tile([128, FPC], F32, tag="pv")
                for kc in range(KC):
                    nc.tensor.matmul(
                        acc, XT[:, kc, 128 * t16:128 * t16 + 128],
                        WV[:, kc, :], start=(kc == 0), stop=(kc == KC - 1))
                nc.vector.tensor_copy(
                    out=VA[:, t16, :, 0:64],
                    in_=acc.rearrange("p (h e) -> p h e", e=64))

            def s_pieces(h, qh, v_list=()):
                """Generator: one S/exp piece per next() (+ V chains)."""
                p, r0 = h // 2, 64 * (h % 2)
                q0 = 1024 * qh
                v_list = list(v_list)
                pts = {}
                for kc in range(8 * (qh + 1)):
                    qs = max(q0, 128 * kc)
                    cols = q0 + 1024 - qs
                    sp = pss.tile([128, 1024], F32, tag="sp")
                    off = 0
                    while off < cols:
                        cw = min(512 - (off % 512), cols - off)
                        nc.tensor.matmul(
                            sp[:, off:off + cw],
                            KT[p][r0:r0 + 64, 128 * kc:128 * kc + 128],
                            QT[p][r0:r0 + 64, qs + off:qs + off + cw],
                            start=True, stop=True)
                        off += cw
                    pt = ptp.tile([128, 1024], BF16, tag="pt")
                    nc.scalar.activation(out=pt[:, 0:cols], in_=sp[:, 0:cols],
                                         func=EXP, scale=CON[:, 0:1])
                    if 128 * kc >= q0:  # diagonal block leads the piece
                        nc.gpsimd.tensor_mul(pt[:, 0:128], pt[:, 0:128], MSK)
                    pts[kc] = (pt, qs)
                    if v_list:
                        v_chain(v_list.pop(0))
                    yield pts

            def pv_chains(h, qh, pts, post_qb=None):
                """Generator: one PV accumulation chain (q-block) per next().
                Chains are sequential per PSUM bank: a start=True in a bank
                wipes still-open chains there, so chain qb fully closes
                before chain qb+1 opens. Each 4-block window drains
                (normalize / transpose / post hook) as soon as it completes."""
                q0 = 1024 * qh
                p = h // 2
                yqs = [psy.tile([128, 2, 128], F32, tag="yq",
                                name=f"yq_{h}_{qh}_{w}") for w in range(4)]
                for qb in range(8):
                    for kc in range(8 * qh + qb + 1):
                        pt, qs = pts[kc]
                        off = 128 * qb + q0 - qs
                        nc.tensor.matmul(
                            yqs[qb // 2][:, qb % 2, 0:65],
                            pt[:, off:off + 128], VA[:, kc, h, :],
                            start=(kc == 0), stop=(kc == 8 * qh + qb))
                    if qb % 2 == 1:   # half-window complete: drain it
                        w = qb // 2
                        yq = yqs[w]
                        rec = recp.tile([128, 2], F32, tag="rec")
                        nc.vector.reciprocal(
                            out=rec,
                            in_=yq[:, :, 64:65].rearrange("p a e -> p (a e)"))
                        for qb4 in range(2):
                            qbg = 8 * qh + 2 * w + qb4
                            if h % 2 == 0:
                                ybufs[(p, qbg)] = ybp.tile(
                                    [128, 2, 64], BF16, tag="yb",
                                    name=f"yb_{p}_{qbg}")
                            nc.vector.tensor_scalar_mul(
                                ybufs[(p, qbg)][:, h % 2, :],
                                yq[:, qb4, 0:64], rec[:, qb4:qb4 + 1])
                        if h % 2 == 1:
                            for qb4 in range(2):
                                qbg = 8 * qh + 2 * w + qb4
                                yb = ybufs.pop((p, qbg))
                                nc.sync.dma_start_transpose(
                                    out=AT[p][:, 128 * qbg:128 * qbg + 128],
                                    in_=yb[:, :, :].rearrange(
                                        "p a e -> p (a e)"))
                            if post_qb is not None:
                                for qb4 in range(2):
                                    post_qb(8 * qh + 2 * w + qb4)
                    yield

            def oproj(t16, split_evict=False):
                ot = otp.tile([128, D], BF16, tag="ot")
                for half in range(2):
                    yo = pso.tile([128, 512], F32, tag="yo")
                    for ft in range(2):
                        nc.tensor.matmul(
                            yo, AT[ft][:, 128 * t16:128 * t16 + 128],
                            WOC[:, ft, 512 * half:512 * half + 512],
                            start=(ft == 0), stop=(ft == 1))
                    if split_evict and half == 1:
                        nc.scalar.activation(
                            out=ot[:, 512 * half:512 * half + 512], in_=yo,
                            func=CPY, scale=CON[:, 1:2])
                    else:
                        nc.vector.tensor_scalar_mul(
                            ot[:, 512 * half:512 * half + 512], yo,
                            CON[:, 1:2])
                nc.sync.dma_start(out=yp_d[128 * t16:128 * t16 + 128, :],
                                  in_=ot)

            def drive(chain_gen, piece_gen, opro=(), ratio=1):
                """Round-robin: PV chains of head h with S pieces of head
                h+1 (and O-proj chunks) until all exhausted."""
                opro = list(opro)
                last = None
                c_done = chain_gen is None
                p_done = piece_gen is None
                while not (c_done and p_done and not opro):
                    if opro:
                        oproj(opro.pop(0))
                    if not c_done:
                        try:
                            next(chain_gen)
                        except StopIteration:
                            c_done = True
                    if not p_done:
                        for _ in range(ratio):
                            try:
                                last = next(piece_gen)
                            except StopIteration:
                                p_done = True
                                break
                return last

            # ---- pipelined schedule ----
            HEADS = [(h, qh) for qh in (0, 1) for h in range(4)]
            chain_gen = None
            pso_cm = None
            for h, qh in HEADS:
                if (h, qh) == (2, 0):
                    # pair-1 rope deferred here so the early DVE queue
                    # (V evicts feed the PV chains) is not blocked by it
                    rope(QT, 1, 0)
                    rope(KT, 1, 1)
                if (h, qh) == (2, 1):
                    # all V chains emitted by end of (1,1): psv -> pso
                    psv_cm.__exit__(None, None, None)
                    pso_cm = tc.tile_pool(name="pso", bufs=2, space="PSUM")
                    pso = pso_cm.__enter__()
                v_list = ()
                if h == 0:
                    v_list = range(8 * qh, 8 * qh + 4)
                elif h == 1:
                    v_list = range(8 * qh + 4, 8 * qh + 8)
                piece_gen = s_pieces(h, qh, v_list=v_list)
                opro = ()
                if (h, qh) == (2, 1):
                    opro = range(0, 4)
                elif (h, qh) == (3, 1):
                    opro = range(4, 8)
                pts = drive(chain_gen, piece_gen, opro, ratio=qh + 1)
                chain_gen = pv_chains(h, qh, pts)
            drive(chain_gen, None)
            # tail: widen the O-proj PSUM pool so the last 8 chunks pipeline
            pso_cm.__exit__(None, None, None)
            psy_cm.__exit__(None, None, None)
            pss_cm.__exit__(None, None, None)
            pso_cm = tc.tile_pool(name="pso2", bufs=8, space="PSUM")
            pso = pso_cm.__enter__()
            for t16 in range(8, 16):
                oproj(t16, split_evict=True)
            pso_cm.__exit__(None, None, None)
            otp_cm.__exit__(None, None, None)
            recp_cm.__exit__(None, None, None)
            ybp_cm.__exit__(None, None, None)
            ptp_cm.__exit__(None, None, None)

    _split_excess_waits(nc)
    return nc


_NC = None
_LAST_INMAPS = None


def _get_nc():
    global _NC
    if _NC is None:
        _NC = _build()
    return _NC


def _ternary_signs(w):
    """Mirror reference ternary_weight: returns (signs in {-1,0,1}, scale)."""
    try:
        import jax
        import jax.numpy as jnp
        cpu = jax.devices("cpu")[0]
        with jax.default_device(cpu):
            wj = jnp.asarray(np.asarray(w, dtype=np.float32))
            scale = jnp.mean(jnp.abs(wj))
            signs = jnp.round(jnp.clip(wj / (scale + 1e-8), -1.0, 1.0))
            return np.asarray(signs, dtype=np.float32), float(scale)
    except Exception:
        w = np.asarray(w, dtype=np.float32)
        scale = np.float32(np.mean(np.abs(w)))
        signs = np.round(np.clip(w / (scale + np.float32(1e-8)), -1.0, 1.0))
        return signs.astype(np.float32), float(scale)


def _rope_tables():
    inv = (1.0 / (10000.0 ** (np.arange(0, HD, 2, dtype=np.float32) / HD))
           ).astype(np.float32)                      # [32]
    t = np.arange(T, dtype=np.float32)
    fr = np.outer(t, inv).astype(np.float32)         # [T, 32]
    cos1 = np.cos(fr).astype(np.float32)
    sin1 = np.sin(fr).astype(np.float32)
    # rows: d in 0..63 (freq d%32), tiled for 2 heads -> 128 rows
    cosd = np.concatenate([cos1, cos1], axis=1).T    # [64, T]
    sind = np.concatenate([sin1, sin1], axis=1).T
    sgn = np.ones((HD, 1), dtype=np.float32)
    sgn[:HD // 2] = -1.0
    cos2 = np.tile(cosd, (2, 1)).astype(NPBF)                # [128, T]
    sins = np.tile(sind * sgn, (2, 1)).astype(NPBF)
    return cos2, sins


def kernel(x, Wq, Wk, Wv, Wo, mask):
    global _LAST_INMAPS
    x = np.asarray(x, dtype=np.float32)
    mask = np.asarray(mask)
    assert np.array_equal(
        np.asarray(mask[0, 0], dtype=np.int32),
        np.tril(np.ones((T, T), dtype=np.int32))), "non-causal mask"

    qs, sq = _ternary_signs(Wq)
    ks, sk = _ternary_signs(Wk)
    vs, sv = _ternary_signs(Wv)
    os_, so = _ternary_signs(Wo)
    cos2, sins = _rope_tables()
    mvals = np.triu(np.ones((128, 128), dtype=np.float32)).astype(NPBF)
    consts = np.zeros((128, 2), dtype=np.float32)
    consts[:, 0] = np.float32(sq) * np.float32(sk) * np.float32(0.125)
    consts[:, 1] = np.float32(sv) * np.float32(so)

    in_maps = []
    for c in range(NCORES):
        b, g = c // 4, c % 4
        fsl = slice(FPC * g, FPC * g + FPC)
        in_maps.append({
            "xt": np.ascontiguousarray(x[b].T).astype(NPBF),
            "wqt": np.ascontiguousarray(qs[fsl].T).astype(NPBF),
            "wkt": np.ascontiguousarray(ks[fsl].T).astype(NPBF),
            "wvt": np.ascontiguousarray(vs[fsl].T).astype(NPBF),
            "woc": np.ascontiguousarray(os_[:, fsl].T).astype(NPBF),
            "cos2": cos2,
            "sins": sins,
            "maskm": mvals,
            "consts": consts,
        })
    _LAST_INMAPS = in_maps

    res = run_bass_kernel_spmd(_get_nc(), in_maps,
                               core_ids=list(range(NCORES)))
    out = np.zeros((B, T, D), dtype=np.float32)
    for b in range(B):
        acc = np.zeros((T, D), dtype=np.float32)
        for g in range(4):
            acc += np.asarray(res.results[4 * b + g]["yp"],
                              dtype=np.float32)
        out[b] = acc
    return out


def bench(trace=True):
    """Re-run last inputs with NTFF tracing; returns BassKernelResults."""
    assert _LAST_INMAPS is not None, "call kernel() first"
    return run_bass_kernel_spmd(_get_nc(), _LAST_INMAPS,
                                core_ids=list(range(NCORES)), trace=trace)


# revision 72
# speedup vs baseline: 1.5050x; 1.0039x over previous
"""BinarySelfAttention Trainium2 kernel (8-core SPMD), v2.

Strategy: shard (batch, head-group): core c -> batch c//4, heads 4*(c%4)..+3.
Each core computes ternary-projected QKV for its 4 heads (bf16 data path,
ternary signs exact in bf16), RoPE, causal attention in S^T orientation
(keys on partitions), a FLIPPED PV matmul (exp(S^T) chunks stationary, V
moving at 65 cols -> half the PE time of the 65-row orientation), per-
partition softmax normalization (no DRAM bounce), DMA-engine transposes of
the normalized y into [feature, T] layout, and a partial output projection
against its Wo column slice. Host sums the 4 bf16 partials per batch in f32.

Ternary scales are folded into the exp() scale (sq*sk/8) and the output
eviction (sv*so), passed as runtime data so the program is input-independent.
"""
import numpy as np
import ml_dtypes

import concourse.bass as bass
import concourse.mybir as mybir
import concourse.tile as tile
from concourse.bass_utils import run_bass_kernel_spmd

F32 = mybir.dt.float32
BF16 = mybir.dt.bfloat16
FP8 = mybir.dt.float8e4
NPBF = ml_dtypes.bfloat16
NPF8 = ml_dtypes.float8_e4m3

B, T, D, H = 2, 2048, 1024, 16
HD = 64            # head dim
HPC = 4            # heads per core
FPC = HPC * HD     # features per core (256)
NCORES = 8
KC = D // 128      # 8 contraction chunks for projections


def _split_excess_waits(nc):
    """walrus wait-slot limits: 1 for most instructions, 0 for the DMA
    transpose, 1 for TensorScalarPtr; hoist excess onto same-queue NoOps."""
    LIMS = {"InstDmaTransposeAnt": 0, "InstTensorScalarPtr": 1}
    n = 0
    for f in nc.m.functions:
        for bb in f.blocks:
            new_insts = []
            for inst in bb.instructions:
                si = getattr(inst, 'sync_info', None)
                lim = LIMS.get(type(inst).__name__, 1)
                if si is not None and si.on_wait and len(si.on_wait) > lim:
                    waits = list(si.on_wait)
                    extra, keep = (waits, []) if lim == 0 else \
                        (waits[:-lim], waits[-lim:])
                    for j, w in enumerate(extra):
                        new_insts.append(mybir.InstNoOp(
                            name=f"{inst.name}-wsplit{j}",
                            engine=inst.engine,
                            sync_info=mybir.SyncInfo(on_wait=[w], on_update=[]),
                            bass_nofuse=True,
                        ))
                        n += 1
                    inst.sync_info = mybir.SyncInfo(
                        on_wait=keep, on_update=si.on_update)
                new_insts.append(inst)
            bb.instructions[:] = new_insts
    return n


def _build():
    nc = bass.Bass("TRN2", target_bir_lowering=False, debug=False,
                   num_devices=NCORES)
    xt_d = nc.dram_tensor("xt", [D, T], BF16, kind="ExternalInput")
    wq_d = nc.dram_tensor("wqt", [D, FPC], BF16, kind="ExternalInput")
    wk_d = nc.dram_tensor("wkt", [D, FPC], BF16, kind="ExternalInput")
    wv_d = nc.dram_tensor("wvt", [D, FPC], BF16, kind="ExternalInput")
    wo_d = nc.dram_tensor("woc", [FPC, D], BF16, kind="ExternalInput")
    cos_d = nc.dram_tensor("cos2", [128, T], BF16, kind="ExternalInput")
    sin_d = nc.dram_tensor("sins", [128, T], BF16, kind="ExternalInput")
    msk_d = nc.dram_tensor("maskm", [128, 128], BF16, kind="ExternalInput")
    con_d = nc.dram_tensor("consts", [128, 2], F32, kind="ExternalInput")
    yp_d = nc.dram_tensor("yp", [T, D], BF16, kind="ExternalOutput")

    EXP = mybir.ActivationFunctionType.Exp
    CPY = mybir.ActivationFunctionType.Copy

    with tile.TileContext(nc) as tc:
        with tc.tile_pool(name="main", bufs=1) as mp:
            CON = mp.tile([128, 2], F32)
            MSK = mp.tile([128, 128], BF16)
            XT = mp.tile([128, KC, T], BF16)
            COS = mp.tile([128, T], BF16)
            SIN = mp.tile([128, T], BF16)
            QT = [mp.tile([128, T], BF16, name=f"qt{i}") for i in range(2)]
            KT = [mp.tile([128, T], BF16, name=f"kt{i}") for i in range(2)]
            VA = mp.tile([128, 16, HPC, 65], BF16)
            AT = [mp.tile([128, T], BF16, name=f"at{i}") for i in range(2)]
            WOC = mp.tile([128, 2, D], BF16)
            ONES = mp.tile([128, 64], BF16)

            # DMA issue order drives the serial DMA device: x chunks first
            # (kc-streaming projections), each ternary weight as ONE batched
            # SWDGE transfer, Wo/consts deferred (needed late).
            WQ = mp.tile([128, KC, FPC], BF16, name="wq")
            WK = mp.tile([128, KC, FPC], BF16, name="wk")
            WV = mp.tile([128, KC, FPC], BF16, name="wv")
            nc.sync.dma_start(
                out=WQ[:, 0:4, :],
                in_=wq_d[0:512, :].rearrange("(kc p) f -> p kc f", p=128))
            nc.scalar.dma_start(
                out=WK[:, 0:4, :],
                in_=wk_d[0:512, :].rearrange("(kc p) f -> p kc f", p=128))
            nc.sync.dma_start(out=XT[:, 0, 0:1024],
                              in_=xt_d[0:128, 0:1024])
            nc.sync.dma_start(out=XT[:, 0, 1024:2048],
                              in_=xt_d[0:128, 1024:2048])
            nc.gpsimd.dma_start(
                out=WQ[:, 4:8, :],
                in_=wq_d[512:1024, :].rearrange("(kc p) f -> p kc f", p=128))
            nc.gpsimd.dma_start(
                out=WK[:, 4:8, :],
                in_=wk_d[512:1024, :].rearrange("(kc p) f -> p kc f", p=128))
            for kc in range(1, KC):
                eng = nc.sync if kc % 2 == 0 else nc.scalar
                eng.dma_start(out=XT[:, kc, :],
                              in_=xt_d[128 * kc:128 * kc + 128, :])
            nc.gpsimd.dma_start(
                out=WV[:, :, :],
                in_=wv_d.rearrange("(kc p) f -> p kc f", p=128))
            nc.scalar.dma_start(out=COS, in_=cos_d[:, :])
            nc.scalar.dma_start(out=SIN, in_=sin_d[:, :])
            nc.sync.dma_start(out=CON, in_=con_d[:, :])
            nc.sync.dma_start(out=MSK, in_=msk_d[:, :])
            nc.vector.memset(ONES, 1.0)
            ones_view = VA[:, :, :, 64:65].rearrange("p a h e -> p (a h e)")
            nc.vector.tensor_copy(out=ones_view, in_=ONES[:, 0:64])
            for ft in range(2):
                nc.scalar.dma_start(out=WOC[:, ft, :],
                                    in_=wo_d[128 * ft:128 * ft + 128, :])

            def proj_pair(dt_i, psqk):
                # kc-streaming Q+K for ONE head pair (8 matmuls/kc matches
                # the x-chunk DMA pace); evictions split Act/DVE
                accs = [psqk.tile([128, 512], F32, tag=f"pq{w}{i}",
                                  name=f"acc{dt_i}{w}{i}")
                        for w in range(2) for i in range(4)]
                for kc in range(KC):
                    for w, wt in enumerate((WQ, WK)):
                        for tch in range(4):
                            nc.tensor.matmul(
                                accs[4 * w + tch],
                                wt[:, kc, 128 * dt_i:128 * dt_i + 128],
                                XT[:, kc, 512 * tch:512 * tch + 512],
                                start=(kc == 0), stop=(kc == KC - 1))
                for w, dest in enumerate((QT, KT)):
                    for tch in range(4):
                        eng = (nc.scalar.copy if tch % 2 == 0
                               else nc.vector.tensor_copy)
                        eng(out=dest[dt_i][:, 512 * tch:512 * tch + 512],
                            in_=accs[4 * w + tch])

            ROT = [mp.tile([128, T], BF16, name=f"rot{i}") for i in range(2)]

            def rope(dest, dt_i, rot_i):
                dst = dest[dt_i]
                rot = ROT[rot_i]
                for g in range(2):
                    b0 = 64 * g
                    nc.gpsimd.dma_start(out=rot[b0:b0 + 32, :],
                                        in_=dst[b0 + 32:b0 + 64, :])
                    nc.gpsimd.dma_start(out=rot[b0 + 32:b0 + 64, :],
                                        in_=dst[b0:b0 + 32, :])
                nc.vector.tensor_mul(rot, rot, SIN)
                nc.vector.tensor_mul(dst, dst, COS)
                nc.vector.tensor_add(dst, dst, rot)

            with tc.tile_pool(name="psqk", bufs=1, space="PSUM") as psqk:
                # pair 0 fully first so attention on heads 0,1 starts early
                proj_pair(0, psqk)
                rope(QT, 0, 0)
                rope(KT, 0, 1)
                proj_pair(1, psqk)

            # ------------- Phase 2: attention (+V proj, +O proj) ---------
            # Software-pipelined heads: head h's PV chains interleave with
            # head h+1's S/exp pieces so the Act engine (exp) stays fed by
            # the in-order PE stream; V chains and O-proj chunks fill
            # leftover PE slack.
            ptp_cm = tc.tile_pool(name="ptp", bufs=42)
            ptp = ptp_cm.__enter__()
            ybp_cm = tc.tile_pool(name="ybp", bufs=28)
            ybp = ybp_cm.__enter__()
            recp_cm = tc.tile_pool(name="recp", bufs=6)
            recp = recp_cm.__enter__()
            otp_cm = tc.tile_pool(name="otp", bufs=8)
            otp = otp_cm.__enter__()
            pss_cm = tc.tile_pool(name="pss", bufs=2, space="PSUM")
            pss = pss_cm.__enter__()
            psy_cm = tc.tile_pool(name="psy", bufs=2, space="PSUM")
            psy = psy_cm.__enter__()
            psv_cm = tc.tile_pool(name="psv", bufs=2, space="PSUM")
            psv = psv_cm.__enter__()

            ybufs = {}
            pso = None

            def v_chain(t16):
                acc = psv.tile([128, FPC], F32, tag="pv")
                for kc in range(KC):
                    nc.tensor.matmul(
                        acc, XT[:, kc, 128 * t16:128 * t16 + 128],
                        WV[:, kc, :], start=(kc == 0), stop=(kc == KC - 1))
                nc.vector.tensor_copy(
                    out=VA[:, t16, :, 0:64],
                    in_=acc.rearrange("p (h e) -> p h e", e=64))

            def s_pieces(h, qh, v_list=()):
                """Generator: one S/exp piece per next() (+ V chains)."""
                p, r0 = h // 2, 64 * (h % 2)
                q0 = 1024 * qh
                v_list = list(v_list)
                pts = {}
                for kc in range(8 * (qh + 1)):
                    qs = max(q0, 128 * kc)
                    cols = q0 + 1024 - qs
                    sp = pss.tile([128, 1024], F32, tag="sp")
                    off = 0
                    while off < cols:
                        cw = min(512 - (off % 512), cols - off)
                        nc.tensor.matmul(
                            sp[:, off:off + cw],
                            KT[p][r0:r0 + 64, 128 * kc:128 * kc + 128],
                            QT[p][r0:r0 + 64, qs + off:qs + off + cw],
                            start=True, stop=True)
                        off += cw
                    pt = ptp.tile([128, 1024], BF16, tag="pt")
                    nc.scalar.activation(out=pt[:, 0:cols], in_=sp[:, 0:cols],
                                         func=EXP, scale=CON[:, 0:1])
                    if 128 * kc >= q0:  # diagonal block leads the piece
                        nc.gpsimd.tensor_mul(pt[:, 0:128], pt[:, 0:128], MSK)
                    pts[kc] = (pt, qs)
                    if v_list:
                        v_chain(v_list.pop(0))
                    yield pts

            def pv_chains(h, qh, pts, post_qb=None):
                """Generator: one PV accumulation chain (q-block) per next().
                Chains are sequential per PSUM bank: a start=True in a bank
                wipes still-open chains there, so chain qb fully closes
                before chain qb+1 opens. Each 4-block window drains
                (normalize / transpose / post hook) as soon as it completes."""
                q0 = 1024 * qh
                p = h // 2
                yqs = [psy.tile([128, 2, 128], F32, tag="yq",
                                name=f"yq_{h}_{qh}_{w}") for w in range(4)]
                for qb in range(8):
                    for kc in range(8 * qh + qb + 1):
                        pt, qs = pts[kc]
                        off = 128 * qb + q0 - qs
                        nc.tensor.matmul(
                            yqs[qb // 2][:, qb % 2, 0:65],
                            pt[:, off:off + 128], VA[:, kc, h, :],
                            start=(kc == 0), stop=(kc == 8 * qh + qb))
                    if qb % 2 == 1:   # half-window complete: drain it
                        w = qb // 2
                        yq = yqs[w]
                        rec = recp.tile([128, 2], F32, tag="rec")
                        nc.vector.reciprocal(
                            out=rec,
                            in_=yq[:, :, 64:65].rearrange("p a e -> p (a e)"))
                        for qb4 in range(2):
                            qbg = 8 * qh + 2 * w + qb4
                            if h % 2 == 0:
                                ybufs[(p, qbg)] = ybp.tile(
                                    [128, 2, 64], BF16, tag="yb",
                                    name=f"yb_{p}_{qbg}")
                            nc.vector.tensor_scalar_mul(
                                ybufs[(p, qbg)][:, h % 2, :],
                                yq[:, qb4, 0:64], rec[:, qb4:qb4 + 1])
                        if h % 2 == 1:
                            for qb4 in range(2):
                                qbg = 8 * qh + 2 * w + qb4
                                yb = ybufs.pop((p, qbg))
                                nc.sync.dma_start_transpose(
                                    out=AT[p][:, 128 * qbg:128 * qbg + 128],
                                    in_=yb[:, :, :].rearrange(
                                        "p a e -> p (a e)"))
                            if post_qb is not None:
                                for qb4 in range(2):
                                    post_qb(8 * qh + 2 * w + qb4)
                    yield

            def oproj(t16, split_evict=False):
                ot = otp.tile([128, D], BF16, tag="ot")
                for half in range(2):
                    yo = pso.tile([128, 512], F32, tag="yo")
                    for ft in range(2):
                        nc.tensor.matmul(
                            yo, AT[ft][:, 128 * t16:128 * t16 + 128],
                            WOC[:, ft, 512 * half:512 * half + 512],
                            start=(ft == 0), stop=(ft == 1))
                    if split_evict and half == 1:
                        nc.scalar.activation(
                            out=ot[:, 512 * half:512 * half + 512], in_=yo,
                            func=CPY, scale=CON[:, 1:2])
                    else:
                        nc.vector.tensor_scalar_mul(
                            ot[:, 512 * half:512 * half + 512], yo,
                            CON[:, 1:2])
                nc.sync.dma_start(out=yp_d[128 * t16:128 * t16 + 128, :],
                                  in_=ot)

            def drive(chain_gen, piece_gen, opro=(), ratio=1):
                """Round-robin: PV chains of head h with S pieces of head
                h+1 (and O-proj chunks) until all exhausted."""
                opro = list(opro)
                last = None
                c_done = chain_gen is None
                p_done = piece_gen is None
                while not (c_done and p_done and not opro):
                    if opro:
                        oproj(opro.pop(0))
                    if not p_done:
                        for _ in range(ratio):
                            try:
                                last = next(piece_gen)
                            except StopIteration:
                                p_done = True
                                break
                    if not c_done:
                        try:
                            next(chain_gen)
                        except StopIteration:
                            c_done = True
                return last

            # ---- pipelined schedule ----
            HEADS = [(h, qh) for qh in (0, 1) for h in range(4)]
            chain_gen = None
            pso_cm = None
            for h, qh in HEADS:
                if (h, qh) == (1, 0):
                    # pair-1 rope deferred here so the early DVE queue
                    # (V evicts feed the PV chains) is not blocked by it
                    rope(QT, 1, 0)
                    rope(KT, 1, 1)
                if (h, qh) == (2, 1):
                    # all V chains emitted by end of (1,1): psv -> pso
                    psv_cm.__exit__(None, None, None)
                    pso_cm = tc.tile_pool(name="pso", bufs=2, space="PSUM")
                    pso = pso_cm.__enter__()
                v_list = ()
                if h == 0:
                    v_list = range(8 * qh, 8 * qh + 4)
                elif h == 1:
                    v_list = range(8 * qh + 4, 8 * qh + 8)
                piece_gen = s_pieces(h, qh, v_list=v_list)
                opro = ()
                if (h, qh) == (2, 1):
                    opro = range(0, 4)
                elif (h, qh) == (3, 1):
                    opro = range(4, 8)
                pts = drive(chain_gen, piece_gen, opro, ratio=qh + 1)
                chain_gen = pv_chains(h, qh, pts)
            drive(chain_gen, None)
            # tail: widen the O-proj PSUM pool so the last 8 chunks pipeline
            pso_cm.__exit__(None, None, None)
            psy_cm.__exit__(None, None, None)
            pss_cm.__exit__(None, None, None)
            pso_cm = tc.tile_pool(name="pso2", bufs=8, space="PSUM")
            pso = pso_cm.__enter__()
            for t16 in range(8, 16):
                oproj(t16, split_evict=True)
            pso_cm.__exit__(None, None, None)
            otp_cm.__exit__(None, None, None)
            recp_cm.__exit__(None, None, None)
            ybp_cm.__exit__(None, None, None)
            ptp_cm.__exit__(None, None, None)

    _split_excess_waits(nc)
    return nc


_NC = None
_LAST_INMAPS = None


def _get_nc():
    global _NC
    if _NC is None:
        _NC = _build()
    return _NC


def _ternary_signs(w):
    """Mirror reference ternary_weight: returns (signs in {-1,0,1}, scale)."""
    try:
        import jax
        import jax.numpy as jnp
        cpu = jax.devices("cpu")[0]
        with jax.default_device(cpu):
            wj = jnp.asarray(np.asarray(w, dtype=np.float32))
            scale = jnp.mean(jnp.abs(wj))
            signs = jnp.round(jnp.clip(wj / (scale + 1e-8), -1.0, 1.0))
            return np.asarray(signs, dtype=np.float32), float(scale)
    except Exception:
        w = np.asarray(w, dtype=np.float32)
        scale = np.float32(np.mean(np.abs(w)))
        signs = np.round(np.clip(w / (scale + np.float32(1e-8)), -1.0, 1.0))
        return signs.astype(np.float32), float(scale)


def _rope_tables():
    inv = (1.0 / (10000.0 ** (np.arange(0, HD, 2, dtype=np.float32) / HD))
           ).astype(np.float32)                      # [32]
    t = np.arange(T, dtype=np.float32)
    fr = np.outer(t, inv).astype(np.float32)         # [T, 32]
    cos1 = np.cos(fr).astype(np.float32)
    sin1 = np.sin(fr).astype(np.float32)
    # rows: d in 0..63 (freq d%32), tiled for 2 heads -> 128 rows
    cosd = np.concatenate([cos1, cos1], axis=1).T    # [64, T]
    sind = np.concatenate([sin1, sin1], axis=1).T
    sgn = np.ones((HD, 1), dtype=np.float32)
    sgn[:HD // 2] = -1.0
    cos2 = np.tile(cosd, (2, 1)).astype(NPBF)                # [128, T]
    sins = np.tile(sind * sgn, (2, 1)).astype(NPBF)
    return cos2, sins


def kernel(x, Wq, Wk, Wv, Wo, mask):
    global _LAST_INMAPS
    x = np.asarray(x, dtype=np.float32)
    mask = np.asarray(mask)
    assert np.array_equal(
        np.asarray(mask[0, 0], dtype=np.int32),
        np.tril(np.ones((T, T), dtype=np.int32))), "non-causal mask"

    qs, sq = _ternary_signs(Wq)
    ks, sk = _ternary_signs(Wk)
    vs, sv = _ternary_signs(Wv)
    os_, so = _ternary_signs(Wo)
    cos2, sins = _rope_tables()
    mvals = np.triu(np.ones((128, 128), dtype=np.float32)).astype(NPBF)
    consts = np.zeros((128, 2), dtype=np.float32)
    consts[:, 0] = np.float32(sq) * np.float32(sk) * np.float32(0.125)
    consts[:, 1] = np.float32(sv) * np.float32(so)

    in_maps = []
    for c in range(NCORES):
        b, g = c // 4, c % 4
        fsl = slice(FPC * g, FPC * g + FPC)
        in_maps.append({
            "xt": np.ascontiguousarray(x[b].T).astype(NPBF),
            "wqt": np.ascontiguousarray(qs[fsl].T).astype(NPBF),
            "wkt": np.ascontiguousarray(ks[fsl].T).astype(NPBF),
            "wvt": np.ascontiguousarray(vs[fsl].T).astype(NPBF),
            "woc": np.ascontiguousarray(os_[:, fsl].T).astype(NPBF),
            "cos2": cos2,
            "sins": sins,
            "maskm": mvals,
            "consts": consts,
        })
    _LAST_INMAPS = in_maps

    res = run_bass_kernel_spmd(_get_nc(), in_maps,
                               core_ids=list(range(NCORES)))
    out = np.zeros((B, T, D), dtype=np.float32)
    for b in range(B):
        acc = np.zeros((T, D), dtype=np.float32)
        for g in range(4):
            acc += np.asarray(res.results[4 * b + g]["yp"],
                              dtype=np.float32)
        out[b] = acc
    return out


def bench(trace=True):
    """Re-run last inputs with NTFF tracing; returns BassKernelResults."""
    assert _LAST_INMAPS is not None, "call kernel() first"
    return run_bass_kernel_spmd(_get_nc(), _LAST_INMAPS,
                                core_ids=list(range(NCORES)), trace=trace)


# revision 80
# speedup vs baseline: 1.5073x; 1.0015x over previous
"""BinarySelfAttention Trainium2 kernel (8-core SPMD), v2.

Strategy: shard (batch, head-group): core c -> batch c//4, heads 4*(c%4)..+3.
Each core computes ternary-projected QKV for its 4 heads (bf16 data path,
ternary signs exact in bf16), RoPE, causal attention in S^T orientation
(keys on partitions), a FLIPPED PV matmul (exp(S^T) chunks stationary, V
moving at 65 cols -> half the PE time of the 65-row orientation), per-
partition softmax normalization (no DRAM bounce), DMA-engine transposes of
the normalized y into [feature, T] layout, and a partial output projection
against its Wo column slice. Host sums the 4 bf16 partials per batch in f32.

Ternary scales are folded into the exp() scale (sq*sk/8) and the output
eviction (sv*so), passed as runtime data so the program is input-independent.
"""
import numpy as np
import ml_dtypes

import concourse.bass as bass
import concourse.mybir as mybir
import concourse.tile as tile
from concourse.bass_utils import run_bass_kernel_spmd

F32 = mybir.dt.float32
BF16 = mybir.dt.bfloat16
FP8 = mybir.dt.float8e4
NPBF = ml_dtypes.bfloat16
NPF8 = ml_dtypes.float8_e4m3

B, T, D, H = 2, 2048, 1024, 16
HD = 64            # head dim
HPC = 4            # heads per core
FPC = HPC * HD     # features per core (256)
NCORES = 8
KC = D // 128      # 8 contraction chunks for projections


def _split_excess_waits(nc):
    """walrus wait-slot limits: 1 for most instructions, 0 for the DMA
    transpose, 1 for TensorScalarPtr; hoist excess onto same-queue NoOps."""
    LIMS = {"InstDmaTransposeAnt": 0, "InstTensorScalarPtr": 1}
    n = 0
    for f in nc.m.functions:
        for bb in f.blocks:
            new_insts = []
            for inst in bb.instructions:
                si = getattr(inst, 'sync_info', None)
                lim = LIMS.get(type(inst).__name__, 1)
                if si is not None and si.on_wait and len(si.on_wait) > lim:
                    waits = list(si.on_wait)
                    extra, keep = (waits, []) if lim == 0 else \
                        (waits[:-lim], waits[-lim:])
                    for j, w in enumerate(extra):
                        new_insts.append(mybir.InstNoOp(
                            name=f"{inst.name}-wsplit{j}",
                            engine=inst.engine,
                            sync_info=mybir.SyncInfo(on_wait=[w], on_update=[]),
                            bass_nofuse=True,
                        ))
                        n += 1
                    inst.sync_info = mybir.SyncInfo(
                        on_wait=keep, on_update=si.on_update)
                new_insts.append(inst)
            bb.instructions[:] = new_insts
    return n


def _build():
    nc = bass.Bass("TRN2", target_bir_lowering=False, debug=False,
                   num_devices=NCORES)
    xt_d = nc.dram_tensor("xt", [D, T], BF16, kind="ExternalInput")
    wq_d = nc.dram_tensor("wqt", [D, FPC], BF16, kind="ExternalInput")
    wk_d = nc.dram_tensor("wkt", [D, FPC], BF16, kind="ExternalInput")
    wv_d = nc.dram_tensor("wvt", [D, FPC], BF16, kind="ExternalInput")
    wo_d = nc.dram_tensor("woc", [FPC, D], BF16, kind="ExternalInput")
    cos_d = nc.dram_tensor("cos2", [128, T], BF16, kind="ExternalInput")
    sin_d = nc.dram_tensor("sins", [128, T], BF16, kind="ExternalInput")
    msk_d = nc.dram_tensor("maskm", [128, 128], BF16, kind="ExternalInput")
    con_d = nc.dram_tensor("consts", [128, 2], F32, kind="ExternalInput")
    yp_d = nc.dram_tensor("yp", [T, D], BF16, kind="ExternalOutput")

    EXP = mybir.ActivationFunctionType.Exp
    CPY = mybir.ActivationFunctionType.Copy

    with tile.TileContext(nc) as tc:
        with tc.tile_pool(name="main", bufs=1) as mp:
            CON = mp.tile([128, 2], F32)
            MSK = mp.tile([128, 128], BF16)
            XT = mp.tile([128, KC, T], BF16)
            COS = mp.tile([128, T], BF16)
            SIN = mp.tile([128, T], BF16)
            QT = [mp.tile([128, T], BF16, name=f"qt{i}") for i in range(2)]
            KT = [mp.tile([128, T], BF16, name=f"kt{i}") for i in range(2)]
            VA = mp.tile([128, 16, HPC, 65], BF16)
            AT = [mp.tile([128, T], BF16, name=f"at{i}") for i in range(2)]
            WOC = mp.tile([128, 2, D], BF16)
            ONES = mp.tile([128, 64], BF16)

            # DMA issue order drives the serial DMA device: x chunks first
            # (kc-streaming projections), each ternary weight as ONE batched
            # SWDGE transfer, Wo/consts deferred (needed late).
            WQ = mp.tile([128, KC, FPC], BF16, name="wq")
            WK = mp.tile([128, KC, FPC], BF16, name="wk")
            WV = mp.tile([128, KC, FPC], BF16, name="wv")
            nc.sync.dma_start(
                out=WQ[:, 0:4, :],
                in_=wq_d[0:512, :].rearrange("(kc p) f -> p kc f", p=128))
            nc.scalar.dma_start(
                out=WK[:, 0:4, :],
                in_=wk_d[0:512, :].rearrange("(kc p) f -> p kc f", p=128))
            nc.sync.dma_start(out=XT[:, 0, 0:1024],
                              in_=xt_d[0:128, 0:1024])
            nc.scalar.dma_start(out=XT[:, 0, 1024:2048],
                              in_=xt_d[0:128, 1024:2048])
            nc.gpsimd.dma_start(
                out=WQ[:, 4:8, :],
                in_=wq_d[512:1024, :].rearrange("(kc p) f -> p kc f", p=128))
            nc.gpsimd.dma_start(
                out=WK[:, 4:8, :],
                in_=wk_d[512:1024, :].rearrange("(kc p) f -> p kc f", p=128))
            for kc in range(1, KC):
                eng = nc.sync if kc % 2 == 0 else nc.scalar
                eng.dma_start(out=XT[:, kc, :],
                              in_=xt_d[128 * kc:128 * kc + 128, :])
            nc.gpsimd.dma_start(
                out=WV[:, :, :],
                in_=wv_d.rearrange("(kc p) f -> p kc f", p=128))
            nc.scalar.dma_start(out=COS, in_=cos_d[:, :])
            nc.scalar.dma_start(out=SIN, in_=sin_d[:, :])
            nc.sync.dma_start(out=CON, in_=con_d[:, :])
            nc.sync.dma_start(out=MSK, in_=msk_d[:, :])
            nc.vector.memset(ONES, 1.0)
            ones_view = VA[:, :, :, 64:65].rearrange("p a h e -> p (a h e)")
            nc.vector.tensor_copy(out=ones_view, in_=ONES[:, 0:64])
            for ft in range(2):
                nc.scalar.dma_start(out=WOC[:, ft, :],
                                    in_=wo_d[128 * ft:128 * ft + 128, :])

            def proj_pair(dt_i, psqk):
                # kc-streaming Q+K for ONE head pair (8 matmuls/kc matches
                # the x-chunk DMA pace); evictions split Act/DVE
                accs = [psqk.tile([128, 512], F32, tag=f"pq{w}{i}",
                                  name=f"acc{dt_i}{w}{i}")
                        for w in range(2) for i in range(4)]
                for kc in range(KC):
                    for w, wt in enumerate((WQ, WK)):
                        for tch in range(4):
                            nc.tensor.matmul(
                                accs[4 * w + tch],
                                wt[:, kc, 128 * dt_i:128 * dt_i + 128],
                                XT[:, kc, 512 * tch:512 * tch + 512],
                                start=(kc == 0), stop=(kc == KC - 1))
                for w, dest in enumerate((QT, KT)):
                    for tch in range(4):
                        eng = (nc.scalar.copy if tch % 2 == 0
                               else nc.vector.tensor_copy)
                        eng(out=dest[dt_i][:, 512 * tch:512 * tch + 512],
                            in_=accs[4 * w + tch])

            ROT = [mp.tile([128, T], BF16, name=f"rot{i}") for i in range(2)]

            def rope(dest, dt_i, rot_i):
                dst = dest[dt_i]
                rot = ROT[rot_i]
                for g in range(2):
                    b0 = 64 * g
                    nc.gpsimd.dma_start(out=rot[b0:b0 + 32, :],
                                        in_=dst[b0 + 32:b0 + 64, :])
                    nc.gpsimd.dma_start(out=rot[b0 + 32:b0 + 64, :],
                                        in_=dst[b0:b0 + 32, :])
                nc.vector.tensor_mul(rot, rot, SIN)
                nc.vector.tensor_mul(dst, dst, COS)
                nc.vector.tensor_add(dst, dst, rot)

            with tc.tile_pool(name="psqk", bufs=1, space="PSUM") as psqk:
                # pair 0 fully first so attention on heads 0,1 starts early
                proj_pair(0, psqk)
                rope(QT, 0, 0)
                rope(KT, 0, 1)
                proj_pair(1, psqk)

            # ------------- Phase 2: attention (+V proj, +O proj) ---------
            # Software-pipelined heads: head h's PV chains interleave with
            # head h+1's S/exp pieces so the Act engine (exp) stays fed by
            # the in-order PE stream; V chains and O-proj chunks fill
            # leftover PE slack.
            ptp_cm = tc.tile_pool(name="ptp", bufs=42)
            ptp = ptp_cm.__enter__()
            ybp_cm = tc.tile_pool(name="ybp", bufs=28)
            ybp = ybp_cm.__enter__()
            recp_cm = tc.tile_pool(name="recp", bufs=6)
            recp = recp_cm.__enter__()
            otp_cm = tc.tile_pool(name="otp", bufs=8)
            otp = otp_cm.__enter__()
            pss_cm = tc.tile_pool(name="pss", bufs=2, space="PSUM")
            pss = pss_cm.__enter__()
            psy_cm = tc.tile_pool(name="psy", bufs=2, space="PSUM")
            psy = psy_cm.__enter__()
            psv_cm = tc.tile_pool(name="psv", bufs=2, space="PSUM")
            psv = psv_cm.__enter__()

            ybufs = {}
            pso = None

            def v_chain(t16):
                acc = psv.tile([128, FPC], F32, tag="pv")
                for kc in range(KC):
                    nc.tensor.matmul(
                        acc, XT[:, kc, 128 * t16:128 * t16 + 128],
                        WV[:, kc, :], start=(kc == 0), stop=(kc == KC - 1))
                nc.vector.tensor_copy(
                    out=VA[:, t16, :, 0:64],
                    in_=acc.rearrange("p (h e) -> p h e", e=64))

            def s_pieces(h, qh, v_list=()):
                """Generator: one S/exp piece per next() (+ V chains)."""
                p, r0 = h // 2, 64 * (h % 2)
                q0 = 1024 * qh
                v_list = list(v_list)
                pts = {}
                for kc in range(8 * (qh + 1)):
                    qs = max(q0, 128 * kc)
                    cols = q0 + 1024 - qs
                    sp = pss.tile([128, 1024], F32, tag="sp")
                    off = 0
                    while off < cols:
                        cw = min(512 - (off % 512), cols - off)
                        nc.tensor.matmul(
                            sp[:, off:off + cw],
                            KT[p][r0:r0 + 64, 128 * kc:128 * kc + 128],
                            QT[p][r0:r0 + 64, qs + off:qs + off + cw],
                            start=True, stop=True)
                        off += cw
                    pt = ptp.tile([128, 1024], BF16, tag="pt")
                    nc.scalar.activation(out=pt[:, 0:cols], in_=sp[:, 0:cols],
                                         func=EXP, scale=CON[:, 0:1])
                    if 128 * kc >= q0:  # diagonal block leads the piece
                        nc.gpsimd.tensor_mul(pt[:, 0:128], pt[:, 0:128], MSK)
                    pts[kc] = (pt, qs)
                    if v_list:
                        v_chain(v_list.pop(0))
                    yield pts

            def pv_chains(h, qh, pts, post_qb=None):
                """Generator: one PV accumulation chain (q-block) per next().
                Chains are sequential per PSUM bank: a start=True in a bank
                wipes still-open chains there, so chain qb fully closes
                before chain qb+1 opens. Each 4-block window drains
                (normalize / transpose / post hook) as soon as it completes."""
                q0 = 1024 * qh
                p = h // 2
                yqs = [psy.tile([128, 2, 128], F32, tag="yq",
                                name=f"yq_{h}_{qh}_{w}") for w in range(4)]
                for qb in range(8):
                    for kc in range(8 * qh + qb + 1):
                        pt, qs = pts[kc]
                        off = 128 * qb + q0 - qs
                        nc.tensor.matmul(
                            yqs[qb // 2][:, qb % 2, 0:65],
                            pt[:, off:off + 128], VA[:, kc, h, :],
                            start=(kc == 0), stop=(kc == 8 * qh + qb))
                    if qb % 2 == 1:   # half-window complete: drain it
                        w = qb // 2
                        yq = yqs[w]
                        rec = recp.tile([128, 2], F32, tag="rec")
                        nc.vector.reciprocal(
                            out=rec,
                            in_=yq[:, :, 64:65].rearrange("p a e -> p (a e)"))
                        for qb4 in range(2):
                            qbg = 8 * qh + 2 * w + qb4
                            if h % 2 == 0:
                                ybufs[(p, qbg)] = ybp.tile(
                                    [128, 2, 64], BF16, tag="yb",
                                    name=f"yb_{p}_{qbg}")
                            nc.vector.tensor_scalar_mul(
                                ybufs[(p, qbg)][:, h % 2, :],
                                yq[:, qb4, 0:64], rec[:, qb4:qb4 + 1])
                        if h % 2 == 1:
                            for qb4 in range(2):
                                qbg = 8 * qh + 2 * w + qb4
                                yb = ybufs.pop((p, qbg))
                                nc.sync.dma_start_transpose(
                                    out=AT[p][:, 128 * qbg:128 * qbg + 128],
                                    in_=yb[:, :, :].rearrange(
                                        "p a e -> p (a e)"))
                            if post_qb is not None:
                                for qb4 in range(2):
                                    post_qb(8 * qh + 2 * w + qb4)
                    yield

            def oproj(t16, split_evict=False):
                ot = otp.tile([128, D], BF16, tag="ot")
                for half in range(2):
                    yo = pso.tile([128, 512], F32, tag="yo")
                    for ft in range(2):
                        nc.tensor.matmul(
                            yo, AT[ft][:, 128 * t16:128 * t16 + 128],
                            WOC[:, ft, 512 * half:512 * half + 512],
                            start=(ft == 0), stop=(ft == 1))
                    if split_evict and half == 1:
                        nc.scalar.activation(
                            out=ot[:, 512 * half:512 * half + 512], in_=yo,
                            func=CPY, scale=CON[:, 1:2])
                    else:
                        nc.vector.tensor_scalar_mul(
                            ot[:, 512 * half:512 * half + 512], yo,
                            CON[:, 1:2])
                nc.sync.dma_start(out=yp_d[128 * t16:128 * t16 + 128, :],
                                  in_=ot)

            def drive(chain_gen, piece_gen, opro=(), ratio=1):
                """Round-robin: PV chains of head h with S pieces of head
                h+1 (and O-proj chunks) until all exhausted."""
                opro = list(opro)
                last = None
                c_done = chain_gen is None
                p_done = piece_gen is None
                while not (c_done and p_done and not opro):
                    if opro:
                        oproj(opro.pop(0))
                    if not p_done:
                        for _ in range(ratio):
                            try:
                                last = next(piece_gen)
                            except StopIteration:
                                p_done = True
                                break
                    if not c_done:
                        try:
                            next(chain_gen)
                        except StopIteration:
                            c_done = True
                return last

            # ---- pipelined schedule ----
            HEADS = [(h, qh) for qh in (0, 1) for h in range(4)]
            chain_gen = None
            pso_cm = None
            for h, qh in HEADS:
                if (h, qh) == (1, 0):
                    # pair-1 rope deferred here so the early DVE queue
                    # (V evicts feed the PV chains) is not blocked by it
                    rope(QT, 1, 0)
                    rope(KT, 1, 1)
                if (h, qh) == (2, 1):
                    # all V chains emitted by end of (1,1): psv -> pso
                    psv_cm.__exit__(None, None, None)
                    pso_cm = tc.tile_pool(name="pso", bufs=2, space="PSUM")
                    pso = pso_cm.__enter__()
                v_list = ()
                if h == 0:
                    v_list = range(8 * qh, 8 * qh + 4)
                elif h == 1:
                    v_list = range(8 * qh + 4, 8 * qh + 8)
                piece_gen = s_pieces(h, qh, v_list=v_list)
                opro = ()
                if (h, qh) == (2, 1):
                    opro = range(0, 4)
                elif (h, qh) == (3, 1):
                    opro = range(4, 8)
                ratio = 3 if (h, qh) == (0, 1) else qh + 1
                pts = drive(chain_gen, piece_gen, opro, ratio=ratio)
                chain_gen = pv_chains(h, qh, pts)
            drive(chain_gen, None)
            # tail: widen the O-proj PSUM pool so the last 8 chunks pipeline
            pso_cm.__exit__(None, None, None)
            psy_cm.__exit__(None, None, None)
            pss_cm.__exit__(None, None, None)
            pso_cm = tc.tile_pool(name="pso2", bufs=8, space="PSUM")
            pso = pso_cm.__enter__()
            for t16 in range(8, 16):
                oproj(t16, split_evict=True)
            pso_cm.__exit__(None, None, None)
            otp_cm.__exit__(None, None, None)
            recp_cm.__exit__(None, None, None)
            ybp_cm.__exit__(None, None, None)
            ptp_cm.__exit__(None, None, None)

    _split_excess_waits(nc)
    return nc


_NC = None
_LAST_INMAPS = None


def _get_nc():
    global _NC
    if _NC is None:
        _NC = _build()
    return _NC


def _ternary_signs(w):
    """Mirror reference ternary_weight: returns (signs in {-1,0,1}, scale)."""
    try:
        import jax
        import jax.numpy as jnp
        cpu = jax.devices("cpu")[0]
        with jax.default_device(cpu):
            wj = jnp.asarray(np.asarray(w, dtype=np.float32))
            scale = jnp.mean(jnp.abs(wj))
            signs = jnp.round(jnp.clip(wj / (scale + 1e-8), -1.0, 1.0))
            return np.asarray(signs, dtype=np.float32), float(scale)
    except Exception:
        w = np.asarray(w, dtype=np.float32)
        scale = np.float32(np.mean(np.abs(w)))
        signs = np.round(np.clip(w / (scale + np.float32(1e-8)), -1.0, 1.0))
        return signs.astype(np.float32), float(scale)


def _rope_tables():
    inv = (1.0 / (10000.0 ** (np.arange(0, HD, 2, dtype=np.float32) / HD))
           ).astype(np.float32)                      # [32]
    t = np.arange(T, dtype=np.float32)
    fr = np.outer(t, inv).astype(np.float32)         # [T, 32]
    cos1 = np.cos(fr).astype(np.float32)
    sin1 = np.sin(fr).astype(np.float32)
    # rows: d in 0..63 (freq d%32), tiled for 2 heads -> 128 rows
    cosd = np.concatenate([cos1, cos1], axis=1).T    # [64, T]
    sind = np.concatenate([sin1, sin1], axis=1).T
    sgn = np.ones((HD, 1), dtype=np.float32)
    sgn[:HD // 2] = -1.0
    cos2 = np.tile(cosd, (2, 1)).astype(NPBF)                # [128, T]
    sins = np.tile(sind * sgn, (2, 1)).astype(NPBF)
    return cos2, sins


def kernel(x, Wq, Wk, Wv, Wo, mask):
    global _LAST_INMAPS
    x = np.asarray(x, dtype=np.float32)
    mask = np.asarray(mask)
    assert np.array_equal(
        np.asarray(mask[0, 0], dtype=np.int32),
        np.tril(np.ones((T, T), dtype=np.int32))), "non-causal mask"

    qs, sq = _ternary_signs(Wq)
    ks, sk = _ternary_signs(Wk)
    vs, sv = _ternary_signs(Wv)
    os_, so = _ternary_signs(Wo)
    cos2, sins = _rope_tables()
    mvals = np.triu(np.ones((128, 128), dtype=np.float32)).astype(NPBF)
    consts = np.zeros((128, 2), dtype=np.float32)
    consts[:, 0] = np.float32(sq) * np.float32(sk) * np.float32(0.125)
    consts[:, 1] = np.float32(sv) * np.float32(so)

    in_maps = []
    for c in range(NCORES):
        b, g = c // 4, c % 4
        fsl = slice(FPC * g, FPC * g + FPC)
        in_maps.append({
            "xt": np.ascontiguousarray(x[b].T).astype(NPBF),
            "wqt": np.ascontiguousarray(qs[fsl].T).astype(NPBF),
            "wkt": np.ascontiguousarray(ks[fsl].T).astype(NPBF),
            "wvt": np.ascontiguousarray(vs[fsl].T).astype(NPBF),
            "woc": np.ascontiguousarray(os_[:, fsl].T).astype(NPBF),
            "cos2": cos2,
            "sins": sins,
            "maskm": mvals,
            "consts": consts,
        })
    _LAST_INMAPS = in_maps

    res = run_bass_kernel_spmd(_get_nc(), in_maps,
                               core_ids=list(range(NCORES)))
    out = np.zeros((B, T, D), dtype=np.float32)
    for b in range(B):
        acc = np.zeros((T, D), dtype=np.float32)
        for g in range(4):
            acc += np.asarray(res.results[4 * b + g]["yp"],
                              dtype=np.float32)
        out[b] = acc
    return out


def bench(trace=True):
    """Re-run last inputs with NTFF tracing; returns BassKernelResults."""
    assert _LAST_INMAPS is not None, "call kernel() first"
    return run_bass_kernel_spmd(_get_nc(), _LAST_INMAPS,
                                core_ids=list(range(NCORES)), trace=trace)
